# revision 32
# baseline (speedup 1.0000x reference)
"""Trainium2 Bass kernel for a cross-attention transformer block.

Sharding: 8 cores = 4 batch rows x 2 query-groups of 512.
Query groups are quarter-interleaved (q-tiles {0,1,4,5} vs {2,3,6,7} of 128
queries) so the SPMD-shared attention chunk plan skips similar amounts of
masked work on every core.

On-device: modality-embedding gather (indirect DMA, host-computed row
indices), layernorms (bn_stats + PE-transpose into [E, tokens] layout with
LN scale/bias fused into the ACT psum->sbuf copy), QKV / attention / MLP
matmuls in bf16 with fp32 accumulation, masked softmax as exp(S) * mask
with a ones-row appended to V producing denominators for free, fused Gelu.
Host computes gather indices (argsort/cumsum), pre-tiles weights into
SBUF-image layouts, and slices/reassembles per-core tensors.

The attention "plan" classifies each (query-window, key-chunk) block of the
time mask as all-zero (skip S/exp/PV entirely), all-one (no mask multiply),
or partial (multiply by the mask tile). It is computed from the actual
input mask (union over cores, since SPMD shares one program), so it is
exact for arbitrary inputs; sorted ages just make it effective.
"""

import numpy as np
import ml_dtypes

import concourse.bass as bass
import concourse.tile as tile
from concourse import bacc, mybir
from concourse.bass_utils import run_bass_kernel_spmd
from concourse.masks import make_identity

dt = mybir.dt
AF = mybir.ActivationFunctionType
ALU = mybir.AluOpType

B = 4
LQ = 512          # queries per core
LK = 1024         # side (key) sequence length
E = 768
H = 12
DH = 64           # head dim
F = 3072
HALF = 512        # occurrences of each modality per row
EC = E // 128     # 6
FC = F // 128     # 24
KC = LK // 128    # 8
QC = LQ // 128    # 4
NQP = 2           # query windows of 256 (plan granularity)
QW = LQ // NQP    # 256
LN_EPS = 1e-5
N_CORES = 8
BF = ml_dtypes.bfloat16

QTILES_G0 = [0, 1, 4, 5]
QTILES_G1 = [2, 3, 6, 7]

_prog_cache = {}

DEBUG_NAMES = ()


def _build_program(plan, ln_ident):
    """plan: NQP x KC ints (0=skip,1=mask,2=full). ln_ident: 3 bools - LN
    weight==1 & bias==0, enabling batched plain transpose-copies."""
    nc = bacc.Bacc("TRN2", target_bir_lowering=False, debug=False,
                   num_devices=N_CORES)

    def din(name, shape, dty=dt.float32):
        return nc.dram_tensor(name, shape, dty, kind="ExternalInput").ap()

    xh = din("xh", [LQ, E])
    embcat = din("embcat", [2 * HALF + 1, E])
    gidx = din("gidx", [LK], dt.int32)
    ageh = din("ageh", [LQ])
    modage = din("modage", [LK])
    qw_img = din("qw_img", [128, EC * E], dt.bfloat16)
    kw_img = din("kw_img", [128, EC * E], dt.bfloat16)
    vw_img = din("vw_img", [128, EC * E], dt.bfloat16)
    cw_img = din("cw_img", [128, EC * E], dt.bfloat16)
    fcw_img = din("fcw_img", [FC, 128, E], dt.bfloat16)
    pwT = din("pwT", [F, E], dt.bfloat16)
    qb = din("qb", [E])
    kb = din("kb", [E])
    vb = din("vb", [E])
    cb = din("cb", [E])
    fcb = din("fcb", [F])
    pb = din("pb", [E])
    ln0w = din("ln0w", [E]); ln0b = din("ln0b", [E])
    ln1w = din("ln1w", [E]); ln1b = din("ln1b", [E])
    ln2w = din("ln2w", [E]); ln2b = din("ln2b", [E])

    out = nc.dram_tensor("out", [LQ, E], dt.float32, kind="ExternalOutput").ap()
    dbg = {}
    def dout(name, shape):
        dbg[name] = nc.dram_tensor("dbg_" + name, shape, dt.float32,
                                   kind="ExternalOutput").ap()
    for nm in DEBUG_NAMES:
        if nm == "x2": dout(nm, [128, QC, E])

    with tile.TileContext(nc) as tc:
        if True:
            # LIFO pool stack: opened in reverse order of release.
            singles = tc.alloc_tile_pool(name="singles", bufs=1)
            pool_cw = tc.alloc_tile_pool(name="pool_cw", bufs=1)
            pool_yall = tc.alloc_tile_pool(name="pool_yall", bufs=1)
            pool_x2 = tc.alloc_tile_pool(name="pool_x2", bufs=1)
            pool_kv = tc.alloc_tile_pool(name="pool_kv", bufs=1)
            pool_qt = tc.alloc_tile_pool(name="pool_qt", bufs=1)
            pool_mask = tc.alloc_tile_pool(name="pool_mask", bufs=1)
            pool_qkv = tc.alloc_tile_pool(name="pool_qkv", bufs=1)
            pool_snT = tc.alloc_tile_pool(name="pool_snT", bufs=1)

            # ---------------- constants / small params ----------------
            ident = singles.tile([128, 128], dt.bfloat16)
            make_identity(nc, ident[:])
            eps_t = singles.tile([128, 1], dt.float32)
            nc.vector.memset(eps_t[:], LN_EPS)

            def col6(name, ap):  # [E] -> [128, EC] per-chunk columns
                t = singles.tile([128, EC], dt.float32, tag=name)
                nc.sync.dma_start(t[:], ap.rearrange("(c p) -> p c", p=128))
                return t

            qb_t = col6("qb", qb); kb_t = col6("kb", kb)
            ln0w_t = col6("ln0w", ln0w); ln0b_t = col6("ln0b", ln0b)
            ln1w_t = col6("ln1w", ln1w); ln1b_t = col6("ln1b", ln1b)
            ln2w_t = col6("ln2w", ln2w); ln2b_t = col6("ln2b", ln2b)
            fcb_t = singles.tile([128, FC], dt.float32)
            nc.sync.dma_start(fcb_t[:], fcb.rearrange("(c p) -> p c", p=128))

            def bcast768(name, ap):  # [E] -> [128, E] partition-broadcast
                t = singles.tile([128, E], dt.float32, tag=name)
                src = bass.AP(tensor=ap.tensor, offset=ap.offset,
                              ap=[[0, 128]] + ap.ap)
                nc.sync.dma_start(t[:], src)
                return t

            vb_t = bcast768("vb", vb)
            cb_t = bcast768("cb", cb)
            pb_t = bcast768("pb", pb)

            age_bc = singles.tile([128, LQ], dt.float32)
            nc.sync.dma_start(age_bc[:], bass.AP(
                tensor=ageh.tensor, offset=ageh.offset, ap=[[0, 128]] + ageh.ap))
            modage_t = singles.tile([128, KC], dt.float32)
            nc.sync.dma_start(modage_t[:], modage.rearrange("(c p) -> p c", p=128))
            gidx_t = singles.tile([128, KC], dt.int32)
            nc.sync.dma_start(gidx_t[:], gidx.rearrange("(c p) -> p c", p=128))

            def wload(pool, name, img):
                t = pool.tile([128, EC, E], dt.bfloat16, tag=name)
                nc.sync.dma_start(t[:], img.rearrange("p (c d) -> p c d", d=E))
                return t

            qwT_t = wload(pool_qkv, "qwT", qw_img)
            kwT_t = wload(pool_qkv, "kwT", kw_img)
            vwT_t = wload(pool_qkv, "vwT", vw_img)
            cwT_t = wload(pool_cw, "cwT", cw_img)

            # ---------------- layernorm -> transposed bf16 ----------------
            def layernorm_T(src_tile, n_chunks, lnw, lnb, identity_ln,
                            dstT, ln_pool, ps_pool, dst_col0=0):
                """src_tile: [128, n_chunks, E] f32 tokens-on-partitions.
                dstT: [128, EC, cols] bf16; writes cols [dst_col0,
                dst_col0+128*n_chunks), LN w/b fused into the copy."""
                mvs = ln_pool.tile([128, n_chunks, 2], dt.float32, tag="ln_mvs")
                for c in range(n_chunks):
                    stats = ln_pool.tile([128, 3, 6], dt.float32, tag="ln_stats")
                    for sg in range(3):
                        nc.vector.bn_stats(out=stats[:, sg, :],
                                           in_=src_tile[:, c, sg * 256:(sg + 1) * 256])
                    nc.vector.bn_aggr(out=mvs[:, c, :], in_=stats[:])
                rstd = ln_pool.tile([128, n_chunks], dt.float32, tag="ln_rstd")
                nc.scalar.activation(out=rstd[:], in_=mvs[:, :, 1], func=AF.Sqrt,
                                     bias=eps_t[:], scale=1.0)
                nc.vector.reciprocal_approx_fast(out=rstd[:], in_=rstd[:])
                for c in range(n_chunks):
                    xhat = ln_pool.tile([128, E], dt.bfloat16, tag="ln_xhat")
                    nc.vector.tensor_scalar(out=xhat[:], in0=src_tile[:, c, :],
                                            scalar1=mvs[:, c, 0:1],
                                            scalar2=rstd[:, c:c + 1],
                                            op0=ALU.subtract, op1=ALU.mult)
                    col = dst_col0 + c * 128
                    if identity_ln:
                        for half in range(2):
                            pt = ps_pool.tile([128, 3, 128], dt.bfloat16,
                                              space="PSUM", tag="ln_tp3")
                            for j in range(3):
                                ec = half * 3 + j
                                nc.tensor.transpose(
                                    pt[:, j, :], xhat[:, ec * 128:(ec + 1) * 128],
                                    ident[:])
                            nc.scalar.activation(
                                out=dstT[:, half * 3:(half + 1) * 3, col:col + 128],
                                in_=pt[:], func=AF.Identity, bias=0.0, scale=1.0)
                    else:
                        for ec in range(EC):
                            pt = ps_pool.tile([128, 128], dt.bfloat16,
                                              space="PSUM", tag="ln_tp")
                            nc.tensor.transpose(
                                pt[:], xhat[:, ec * 128:(ec + 1) * 128], ident[:])
                            nc.scalar.activation(
                                out=dstT[:, ec, col:col + 128],
                                in_=pt[:], func=AF.Identity,
                                bias=lnb[:, ec:ec + 1], scale=lnw[:, ec:ec + 1])

            # ================= phase A: gather + LN0 -> snT =================
            snT = pool_snT.tile([128, EC, LK], dt.bfloat16)
            with tc.tile_pool(name="phA", bufs=2) as phA, \
                 tc.tile_pool(name="phA_ps", bufs=2, space="PSUM") as phA_ps:
                mxall = phA.tile([128, KC, E], dt.float32, tag="mxall", bufs=1)
                for kc in range(KC):
                    nc.gpsimd.indirect_dma_start(
                        out=mxall[:, kc, :], out_offset=None, in_=embcat,
                        in_offset=bass.IndirectOffsetOnAxis(
                            ap=gidx_t[:, kc:kc + 1], axis=0))
                layernorm_T(mxall[:], KC, ln0w_t, ln0b_t, ln_ident[0],
                            snT, phA, phA_ps)

            # ================= phase B: K^T and V_aug =================
            KT = pool_kv.tile([128, EC, LK], dt.bfloat16)
            Vaug = pool_kv.tile([128, KC, H * (DH + 1)], dt.bfloat16)
            nc.vector.memset(
                Vaug[:].rearrange("p c (h x) -> p c h x", x=DH + 1)[:, :, :, DH:DH + 1],
                1.0)
            with tc.tile_pool(name="phB_ps", bufs=3, space="PSUM") as phB_ps:
                for dc in range(EC):
                    for ks in range(2):
                        pk = phB_ps.tile([128, 512], dt.float32, space="PSUM",
                                         tag="pk")
                        for ec in range(EC):
                            nc.tensor.matmul(
                                pk[:], kwT_t[:, ec, dc * 128:(dc + 1) * 128],
                                snT[:, ec, ks * 512:(ks + 1) * 512],
                                start=(ec == 0), stop=(ec == EC - 1))
                        nc.vector.tensor_scalar_add(
                            KT[:, dc, ks * 512:(ks + 1) * 512], pk[:],
                            kb_t[:, dc:dc + 1])
                for kc in range(KC):
                    for hf in range(2):
                        pv = phB_ps.tile([128, 384], dt.float32, space="PSUM",
                                         tag="pv")
                        for ec in range(EC):
                            nc.tensor.matmul(
                                pv[:], snT[:, ec, kc * 128:(kc + 1) * 128],
                                vwT_t[:, ec, hf * 384:(hf + 1) * 384],
                                start=(ec == 0), stop=(ec == EC - 1))
                        dstv = Vaug[:, kc, :].rearrange(
                            "p (h x) -> p h x", x=DH + 1)[:, hf * 6:(hf + 1) * 6, 0:DH]
                        nc.vector.scalar_tensor_tensor(
                            out=dstv, in0=pv[:].rearrange("p (h x) -> p h x", x=DH),
                            scalar=1.0,
                            in1=vb_t[:, hf * 384:(hf + 1) * 384].rearrange(
                                "p (h x) -> p h x", x=DH),
                            op0=ALU.mult, op1=ALU.add)
            pool_snT.release()

            # ================= phase C: x load, LN1 -> xnT, Q^T =============
            QT = pool_qt.tile([128, EC, LQ], dt.bfloat16)
            with tc.tile_pool(name="phC", bufs=2) as phC, \
                 tc.tile_pool(name="phC_ps", bufs=3, space="PSUM") as phC_ps:
                xc = phC.tile([128, QC, E], dt.float32, tag="xc", bufs=1)
                nc.sync.dma_start(xc[:], xh.rearrange("(c p) e -> p c e", p=128))
                xnT = phC.tile([128, EC, LQ], dt.bfloat16, tag="xnT", bufs=1)
                layernorm_T(xc[:], QC, ln1w_t, ln1b_t, ln_ident[1], xnT,
                            phC, phC_ps)
                for dc in range(EC):
                    pq = phC_ps.tile([128, 512], dt.float32, space="PSUM", tag="pq")
                    for ec in range(EC):
                        nc.tensor.matmul(
                            pq[:], qwT_t[:, ec, dc * 128:(dc + 1) * 128],
                            xnT[:, ec, :],
                            start=(ec == 0), stop=(ec == EC - 1))
                    nc.vector.tensor_scalar_add(QT[:, dc, :], pq[:],
                                                qb_t[:, dc:dc + 1])
            pool_qkv.release()

            # ================= mask =================
            need_mask = [any(plan[qp][kc] == 1 for qp in range(NQP))
                         for kc in range(KC)]
            mask = pool_mask.tile([128, KC, LQ], dt.bfloat16)
            nonneg = singles.tile([128, KC], dt.float32)
            nc.vector.tensor_scalar(out=nonneg[:], in0=modage_t[:], scalar1=0.0,
                                    scalar2=None, op0=ALU.is_ge)
            for kc in range(KC):
                if not need_mask[kc]:
                    continue
                nc.vector.tensor_scalar(out=mask[:, kc, :], in0=age_bc[:],
                                        scalar1=modage_t[:, kc:kc + 1],
                                        scalar2=None, op0=ALU.is_ge)
                nc.vector.tensor_scalar_mul(mask[:, kc, :], mask[:, kc, :],
                                            nonneg[:, kc:kc + 1])

            # ================= phase D: attention =================
            # Per-kc schedule shared by every head: merge the two query
            # windows into one wide op when both are live and have matching
            # accumulation state; per-window ops otherwise.
            last_live = [max((kc for kc in range(KC) if plan[qp][kc] != 0),
                             default=-1) for qp in range(NQP)]
            sched = []  # (kc, c0, c1, start, stop, mask_slices)
            seen = [False] * NQP
            for kc in range(KC):
                lv = [qp for qp in range(NQP) if plan[qp][kc] != 0]
                if not lv:
                    continue
                if len(lv) == 2 and seen[0] == seen[1]:
                    groups = [(0, 2 * QW, lv)]
                else:
                    groups = [(qp * QW, (qp + 1) * QW, [qp]) for qp in lv]
                for c0, c1, qps in groups:
                    msl = []
                    if all(plan[qp][kc] == 1 for qp in qps):
                        msl = [(c0, c1)]
                    else:
                        msl = [(qp * QW, (qp + 1) * QW) for qp in qps
                               if plan[qp][kc] == 1]
                    sched.append((kc, c0, c1, not seen[qps[0]],
                                  all(kc == last_live[qp] for qp in qps), msl))
                for qp in lv:
                    seen[qp] = True

            Yall = pool_yall.tile([128, EC, LQ], dt.bfloat16)
            with tc.tile_pool(name="phD", bufs=6) as phD, \
                 tc.tile_pool(name="phD_ps", bufs=4, space="PSUM") as phD_ps, \
                 tc.tile_pool(name="phD_psy", bufs=3, space="PSUM") as phD_psy:
                for hp in range(H // 2):
                    dc = hp
                    pys = []
                    for hi in range(2):
                        py = phD_psy.tile([128, LQ], dt.float32, space="PSUM",
                                          tag="py", name=f"py_{hp}_{hi}")
                        pys.append(py)
                    for kc, c0, c1, st, sp, msl in sched:
                        pts = []
                        for hi in range(2):
                            p0 = hi * DH
                            ps = phD_ps.tile([128, 2 * QW], dt.float32,
                                             space="PSUM", tag="ps")
                            nc.tensor.matmul(
                                ps[:, 0:c1 - c0],
                                KT[p0:p0 + DH, dc, kc * 128:(kc + 1) * 128],
                                QT[p0:p0 + DH, dc, c0:c1],
                                start=True, stop=True, skip_group_check=True)
                            pt = phD.tile([128, 2 * QW], dt.bfloat16, tag="pt")
                            nc.scalar.activation(out=pt[:, 0:c1 - c0],
                                                 in_=ps[:, 0:c1 - c0],
                                                 func=AF.Exp, bias=0.0, scale=1.0)
                            pts.append(pt)
                        for hi in range(2):
                            for m0, m1 in msl:
                                nc.gpsimd.tensor_tensor(
                                    out=pts[hi][:, m0 - c0:m1 - c0],
                                    in0=pts[hi][:, m0 - c0:m1 - c0],
                                    in1=mask[:, kc, m0:m1],
                                    op=ALU.mult)
                        for hi in range(2):
                            h = 2 * hp + hi
                            nc.tensor.matmul(
                                pys[hi][0:DH + 1, c0:c1],
                                Vaug[:, kc, h * (DH + 1):(h + 1) * (DH + 1)],
                                pts[hi][:, 0:c1 - c0],
                                start=st, stop=sp, skip_group_check=True)
                    for hi in range(2):
                        py = pys[hi]
                        rec = phD.tile([1, LQ], dt.float32, tag="rec")
                        nc.vector.tensor_scalar_add(rec[:], py[DH:DH + 1, :],
                                                    1e-30)
                        nc.vector.reciprocal_approx_fast(out=rec[:], in_=rec[:])
                        recb = phD.tile([DH, LQ], dt.float32, tag="recb")
                        nc.gpsimd.partition_broadcast(recb[:], rec[:])
                        nc.vector.tensor_mul(
                            out=Yall[hi * DH:(hi + 1) * DH, dc, :],
                            in0=py[0:DH, :], in1=recb[:])
            pool_mask.release()
            pool_qt.release()
            pool_kv.release()

            # ============ phase E: c-proj + residual, LN2 -> h1nT ============
            x2 = pool_x2.tile([128, QC, E], dt.float32)
            with tc.tile_pool(name="phE", bufs=1) as phE, \
                 tc.tile_pool(name="phE_ps", bufs=4, space="PSUM") as phE_ps:
                xe = phE.tile([128, QC, E], dt.float32, tag="xe")
                nc.sync.dma_start(xe[:], xh.rearrange("(c p) e -> p c e", p=128))
                for qc in range(QC):
                    for hf in range(2):
                        pc = phE_ps.tile([128, 384], dt.float32, space="PSUM",
                                         tag="pc")
                        for ec in range(EC):
                            nc.tensor.matmul(
                                pc[:], Yall[:, ec, qc * 128:(qc + 1) * 128],
                                cwT_t[:, ec, hf * 384:(hf + 1) * 384],
                                start=(ec == 0), stop=(ec == EC - 1))
                        sl = slice(hf * 384, (hf + 1) * 384)
                        nc.vector.scalar_tensor_tensor(
                            out=x2[:, qc, sl], in0=pc[:], scalar=1.0,
                            in1=cb_t[:, sl], op0=ALU.mult, op1=ALU.add)
                    nc.vector.tensor_add(out=x2[:, qc, :], in0=x2[:, qc, :],
                                         in1=xe[:, qc, :])
            if "x2" in dbg:
                nc.sync.dma_start(dbg["x2"], x2[:])

            pool_h1 = tc.alloc_tile_pool(name="pool_h1", bufs=1)
            h1nT = pool_h1.tile([128, EC, LQ], dt.bfloat16)
            with tc.tile_pool(name="phE2", bufs=2) as phE2, \
                 tc.tile_pool(name="phE2_ps", bufs=3, space="PSUM") as phE2_ps:
                layernorm_T(x2[:], QC, ln2w_t, ln2b_t, ln_ident[2], h1nT,
                            phE2, phE2_ps)

            # ================= phase F: MLP =================
            pool_hT = tc.alloc_tile_pool(name="pool_hT", bufs=1)
            hT = pool_hT.tile([128, FC, LQ], dt.bfloat16)
            with tc.tile_pool(name="phF", bufs=3) as phF, \
                 tc.tile_pool(name="phF_ps", bufs=2, space="PSUM") as phF_ps:
                for f in range(FC):
                    fw = phF.tile([128, EC, 128], dt.bfloat16, tag="fw")
                    nc.sync.dma_start(fw[:], fcw_img[f]
                                      .rearrange("p (c x) -> p c x", c=EC))
                    ph = phF_ps.tile([128, LQ], dt.float32, space="PSUM", tag="ph")
                    for ec in range(EC):
                        nc.tensor.matmul(ph[:], fw[:, ec, :], h1nT[:, ec, :],
                                         start=(ec == 0), stop=(ec == EC - 1))
                    nc.scalar.activation(out=hT[:, f, :], in_=ph[:], func=AF.Gelu,
                                         bias=fcb_t[:, f:f + 1], scale=1.0)

            with tc.tile_pool(name="phG", bufs=3) as phG, \
                 tc.tile_pool(name="phG_ps", bufs=8, space="PSUM") as phG_ps, \
                 tc.tile_pool(name="phG_out", bufs=2) as phG_out:
                pps = [phG_ps.tile([128, 384], dt.float32, space="PSUM", tag="pp",
                                   name=f"pp{i}")
                       for i in range(2 * QC)]
                for f in range(FC):
                    pw = phG.tile([128, E], dt.bfloat16, tag="pw")
                    nc.sync.dma_start(pw[:], pwT[f * 128:(f + 1) * 128, :])
                    for qc in range(QC):
                        for hf in range(2):
                            nc.tensor.matmul(
                                pps[qc * 2 + hf][:],
                                hT[:, f, qc * 128:(qc + 1) * 128],
                                pw[:, hf * 384:(hf + 1) * 384],
                                start=(f == 0), stop=(f == FC - 1),
                                skip_group_check=True)
                for qc in range(QC):
                    ot = phG_out.tile([128, E], dt.float32, tag="ot")
                    for hf in range(2):
                        sl = slice(hf * 384, (hf + 1) * 384)
                        nc.vector.scalar_tensor_tensor(
                            out=ot[:, sl], in0=pps[qc * 2 + hf][:], scalar=1.0,
                            in1=pb_t[:, sl], op0=ALU.mult, op1=ALU.add)
                    nc.vector.tensor_add(out=ot[:], in0=ot[:], in1=x2[:, qc, :])
                    nc.sync.dma_start(
                        out.rearrange("(c p) e -> p c e", p=128)[:, qc, :], ot[:])

            pool_hT.release()
            pool_h1.release()
            pool_x2.release()
            pool_yall.release()
            pool_cw.release()
            singles.release()

    nc.compile()
    return nc


def _to_img(wT):
    """[E, D] (e-major) -> SBUF image [128, EC*D]: img[p, c*D+d] = wT[c*128+p, d]."""
    Ei, D = wT.shape
    return np.ascontiguousarray(
        wT.reshape(Ei // 128, 128, D).transpose(1, 0, 2).reshape(128, -1))


def _host_prep(x, age, mod_idx, mod_age, mod2_emb, mod3_emb,
               ln0_w, ln0_b, ln1_w, ln1_b, ln2_w, ln2_b,
               q_w, q_b, k_w, k_b, v_w, v_b, c_w, c_b,
               fc_w, fc_b, proj_w, proj_b):
    f32 = np.float32
    x = np.asarray(x, f32); age = np.asarray(age, f32)
    mod_idx = np.asarray(mod_idx); mod_age = np.asarray(mod_age, f32)
    mod2_emb = np.asarray(mod2_emb, f32); mod3_emb = np.asarray(mod3_emb, f32)

    scale = np.float32(DH) ** -0.5
    qw_img = _to_img(np.asarray(q_w, f32).T * scale).astype(BF)
    kw_img = _to_img(np.asarray(k_w, f32).T).astype(BF)
    vw_img = _to_img(np.asarray(v_w, f32).T).astype(BF)
    cw_img = _to_img(np.asarray(c_w, f32).T).astype(BF)
    fcw_img = np.ascontiguousarray(
        np.asarray(fc_w, f32).T.reshape(EC, 128, FC, 128)
        .transpose(2, 1, 0, 3).reshape(FC, 128, E)).astype(BF)
    pwT = np.ascontiguousarray(np.asarray(proj_w, f32).T).astype(BF)
    qb2 = np.asarray(q_b, f32) * scale

    lnp = [np.asarray(a, f32) for a in
           (ln0_w, ln0_b, ln1_w, ln1_b, ln2_w, ln2_b)]
    ln_ident = tuple(
        bool(np.all(lnp[2 * i] == 1.0) and np.all(lnp[2 * i + 1] == 0.0))
        for i in range(3))

    shared = dict(
        qw_img=qw_img, kw_img=kw_img, vw_img=vw_img, cw_img=cw_img,
        fcw_img=fcw_img, pwT=pwT,
        qb=qb2, kb=np.asarray(k_b, f32), vb=np.asarray(v_b, f32),
        cb=np.asarray(c_b, f32), fcb=np.asarray(fc_b, f32),
        pb=np.asarray(proj_b, f32),
        ln0w=lnp[0], ln0b=lnp[1], ln1w=lnp[2], ln1b=lnp[3],
        ln2w=lnp[4], ln2b=lnp[5],
    )

    qrows = {0: np.array([t * 128 + i for t in QTILES_G0 for i in range(128)]),
             1: np.array([t * 128 + i for t in QTILES_G1 for i in range(128)])}

    in_maps = []
    plan_counts = np.zeros((NQP, KC, 3), dtype=np.int64)
    for core in range(N_CORES):
        b, g = core // 2, core % 2
        rows = qrows[g]
        order = np.argsort(mod_age[b], kind="stable")
        s_idx = np.asarray(mod_idx[b])[order]
        m2 = s_idx == 2
        m3 = s_idx == 3
        occ2 = np.clip(np.cumsum(m2) - 1, 0, HALF - 1)
        occ3 = np.clip(np.cumsum(m3) - 1, 0, HALF - 1)
        gi = np.full(LK, 2 * HALF, dtype=np.int32)
        gi[m2] = occ2[m2]
        gi[m3] = HALF + occ3[m3]
        embcat = np.concatenate([
            mod2_emb[b * HALF:(b + 1) * HALF],
            mod3_emb[b * HALF:(b + 1) * HALF],
            np.zeros((1, E), f32)], axis=0)
        agec = age[b][rows]
        mk = (agec[None, :] >= mod_age[b][:, None]) & (mod_age[b] >= 0.0)[:, None]
        for qp in range(NQP):
            sub = mk[:, qp * QW:(qp + 1) * QW]
            for kc in range(KC):
                blk = sub[kc * 128:(kc + 1) * 128]
                if not blk.any():
                    plan_counts[qp, kc, 0] += 1
                elif blk.all():
                    plan_counts[qp, kc, 2] += 1
                else:
                    plan_counts[qp, kc, 1] += 1
        in_maps.append(dict(
            xh=np.ascontiguousarray(x[b][rows]),
            embcat=embcat, gidx=gi,
            ageh=np.ascontiguousarray(agec),
            modage=np.ascontiguousarray(mod_age[b]),
            **shared))

    plan = []
    for qp in range(NQP):
        row = []
        for kc in range(KC):
            z, p, fl = plan_counts[qp, kc]
            if z == N_CORES:
                row.append(0)
            elif fl == N_CORES:
                row.append(2)
            else:
                row.append(1)
        if all(a == 0 for a in row):
            row[0] = 1
        plan.append(tuple(row))
    return in_maps, (tuple(plan), ln_ident), qrows


def _run(inputs, trace):
    in_maps, key, qrows = _host_prep(**inputs)
    if key not in _prog_cache:
        _prog_cache[key] = _build_program(*key)
    nc = _prog_cache[key]
    res = run_bass_kernel_spmd(nc, in_maps, core_ids=list(range(N_CORES)),
                               trace=trace)
    out = np.empty((B, 2 * LQ, E), dtype=np.float32)
    for core in range(N_CORES):
        b, g = core // 2, core % 2
        out[b, qrows[g]] = res.results[core]["out"]
    return out, res


def kernel(**inputs):
    return _run(inputs, trace=False)[0]


def run_traced(**inputs):
    return _run(inputs, trace=True)


# revision 36
# speedup vs baseline: 1.4038x; 1.4038x over previous
"""Trainium2 Bass kernel for a cross-attention transformer block.

Sharding: 8 cores = 4 batch rows x 2 query-groups of 512.
Query groups are quarter-interleaved (q-tiles {0,1,4,5} vs {2,3,6,7} of 128
queries) so the SPMD-shared attention chunk plan skips similar amounts of
masked work on every core.

On-device: modality-embedding gather (indirect DMA, host-computed row
indices), layernorms (bn_stats + PE-transpose into [E, tokens] layout with
LN scale/bias fused into the ACT psum->sbuf copy), QKV / attention / MLP
matmuls in bf16 with fp32 accumulation, masked softmax as exp(S) * mask
with a ones-row appended to V producing denominators for free, fused Gelu.
Host computes gather indices (argsort/cumsum), pre-tiles weights into
SBUF-image layouts, and slices/reassembles per-core tensors.

The attention "plan" classifies each (query-window, key-chunk) block of the
time mask as all-zero (skip S/exp/PV entirely), all-one (no mask multiply),
or partial (multiply by the mask tile). It is computed from the actual
input mask (union over cores, since SPMD shares one program), so it is
exact for arbitrary inputs; sorted ages just make it effective.
"""

import numpy as np
import ml_dtypes

import concourse.bass as bass
import concourse.tile as tile
from concourse import bacc, mybir
from concourse.bass_utils import run_bass_kernel_spmd
from concourse.masks import make_identity

dt = mybir.dt
AF = mybir.ActivationFunctionType
ALU = mybir.AluOpType

B = 4
LQ = 512          # queries per core
LK = 1024         # side (key) sequence length
E = 768
H = 12
DH = 64           # head dim
F = 3072
HALF = 512        # occurrences of each modality per row
EC = E // 128     # 6
FC = F // 128     # 24
KC = LK // 128    # 8
QC = LQ // 128    # 4
NQP = 2           # query windows of 256 (plan granularity)
QW = LQ // NQP    # 256
LN_EPS = 1e-5
N_CORES = 8
BF = ml_dtypes.bfloat16

QTILES_G0 = [0, 1, 4, 5]
QTILES_G1 = [2, 3, 6, 7]

_prog_cache = {}

DEBUG_NAMES = ()


def _build_program(plan, ln_ident):
    """plan: NQP x KC ints (0=skip,1=mask,2=full). ln_ident: 3 bools - LN
    weight==1 & bias==0, enabling batched plain transpose-copies."""
    nc = bacc.Bacc("TRN2", target_bir_lowering=False, debug=False,
                   num_devices=N_CORES)

    def din(name, shape, dty=dt.float32):
        return nc.dram_tensor(name, shape, dty, kind="ExternalInput").ap()

    xh = din("xh", [LQ, E])
    embcat = din("embcat", [2 * HALF + 1, E])
    gidx = din("gidx", [LK], dt.int32)
    ageh = din("ageh", [LQ])
    modage = din("modage", [LK])
    qw_img = din("qw_img", [128, EC * E], dt.bfloat16)
    kw_img = din("kw_img", [128, EC * E], dt.bfloat16)
    vw_img = din("vw_img", [128, EC * E], dt.bfloat16)
    cw_img = din("cw_img", [128, EC * E], dt.bfloat16)
    fcw_img = din("fcw_img", [FC, 128, E], dt.bfloat16)
    pwT = din("pwT", [F, E], dt.bfloat16)
    qb = din("qb", [E])
    kb = din("kb", [E])
    vb = din("vb", [E])
    cb = din("cb", [E])
    fcb = din("fcb", [F])
    pb = din("pb", [E])
    ln0w = din("ln0w", [E]); ln0b = din("ln0b", [E])
    ln1w = din("ln1w", [E]); ln1b = din("ln1b", [E])
    ln2w = din("ln2w", [E]); ln2b = din("ln2b", [E])

    out = nc.dram_tensor("out", [LQ, E], dt.float32, kind="ExternalOutput").ap()
    dbg = {}
    def dout(name, shape, dty=dt.bfloat16):
        dbg[name] = nc.dram_tensor("dbg_" + name, shape, dty,
                                   kind="ExternalOutput").ap()
    for nm in DEBUG_NAMES:
        if nm == "x2": dout(nm, [128, QC, E], dt.float32)
        if nm == "snT": dout(nm, [128, EC, LK])
        if nm == "KT": dout(nm, [128, EC, LK])
        if nm == "QT": dout(nm, [128, EC, LQ])
        if nm == "Yall": dout(nm, [128, EC, LQ])

    with tile.TileContext(nc) as tc:
        if True:
            # LIFO pool stack: opened in reverse order of release.
            singles = tc.alloc_tile_pool(name="singles", bufs=1)
            pool_cw = tc.alloc_tile_pool(name="pool_cw", bufs=1)
            pool_yall = tc.alloc_tile_pool(name="pool_yall", bufs=1)
            pool_x2 = tc.alloc_tile_pool(name="pool_x2", bufs=1)
            pool_kv = tc.alloc_tile_pool(name="pool_kv", bufs=1)
            pool_qt = tc.alloc_tile_pool(name="pool_qt", bufs=1)
            pool_mask = tc.alloc_tile_pool(name="pool_mask", bufs=1)
            pool_qkv = tc.alloc_tile_pool(name="pool_qkv", bufs=1)
            pool_snT = tc.alloc_tile_pool(name="pool_snT", bufs=1)

            # ---------------- constants / small params ----------------
            ident = singles.tile([128, 128], dt.bfloat16)
            make_identity(nc, ident[:])
            eps_t = singles.tile([128, 1], dt.float32)
            nc.vector.memset(eps_t[:], LN_EPS)

            def col6(name, ap):  # [E] -> [128, EC] per-chunk columns
                t = singles.tile([128, EC], dt.float32, tag=name)
                nc.sync.dma_start(t[:], ap.rearrange("(c p) -> p c", p=128))
                return t

            qb_t = col6("qb", qb); kb_t = col6("kb", kb)
            ln0w_t = col6("ln0w", ln0w); ln0b_t = col6("ln0b", ln0b)
            ln1w_t = col6("ln1w", ln1w); ln1b_t = col6("ln1b", ln1b)
            ln2w_t = col6("ln2w", ln2w); ln2b_t = col6("ln2b", ln2b)
            fcb_t = singles.tile([128, FC], dt.float32)
            nc.sync.dma_start(fcb_t[:], fcb.rearrange("(c p) -> p c", p=128))

            def bcast768(name, ap):  # [E] -> [128, E] partition-broadcast
                t = singles.tile([128, E], dt.float32, tag=name)
                src = bass.AP(tensor=ap.tensor, offset=ap.offset,
                              ap=[[0, 128]] + ap.ap)
                nc.sync.dma_start(t[:], src)
                return t

            vb_t = bcast768("vb", vb)
            cb_t = bcast768("cb", cb)
            pb_t = bcast768("pb", pb)

            age_bc = singles.tile([128, LQ], dt.float32)
            nc.sync.dma_start(age_bc[:], bass.AP(
                tensor=ageh.tensor, offset=ageh.offset, ap=[[0, 128]] + ageh.ap))
            modage_t = singles.tile([128, KC], dt.float32)
            nc.sync.dma_start(modage_t[:], modage.rearrange("(c p) -> p c", p=128))
            gidx_t = singles.tile([128, KC], dt.int32)
            nc.sync.dma_start(gidx_t[:], gidx.rearrange("(c p) -> p c", p=128))

            def wload(pool, name, img):
                t = pool.tile([128, EC, E], dt.bfloat16, tag=name)
                nc.sync.dma_start(t[:], img.rearrange("p (c d) -> p c d", d=E))
                return t

            qwT_t = wload(pool_qkv, "qwT", qw_img)
            kwT_t = wload(pool_qkv, "kwT", kw_img)
            vwT_t = wload(pool_qkv, "vwT", vw_img)
            cwT_t = wload(pool_cw, "cwT", cw_img)

            # ---------------- layernorm -> transposed bf16 ----------------
            def layernorm_T(src_tile, n_chunks, lnw, lnb, identity_ln,
                            dstT, ln_pool, ps_pool, dst_col0=0):
                """src_tile: [128, n_chunks, E] f32 tokens-on-partitions.
                dstT: [128, EC, cols] bf16; writes cols [dst_col0,
                dst_col0+128*n_chunks), LN w/b fused into the copy."""
                mvs = ln_pool.tile([128, n_chunks, 2], dt.float32, tag="ln_mvs")
                for c in range(n_chunks):
                    stats = ln_pool.tile([128, 3, 6], dt.float32, tag="ln_stats")
                    for sg in range(3):
                        nc.vector.bn_stats(out=stats[:, sg, :],
                                           in_=src_tile[:, c, sg * 256:(sg + 1) * 256])
                    nc.vector.bn_aggr(out=mvs[:, c, :], in_=stats[:])
                rstd = ln_pool.tile([128, n_chunks], dt.float32, tag="ln_rstd")
                nc.scalar.activation(out=rstd[:], in_=mvs[:, :, 1], func=AF.Sqrt,
                                     bias=eps_t[:], scale=1.0)
                nc.vector.reciprocal_approx_fast(out=rstd[:], in_=rstd[:])
                for c in range(n_chunks):
                    xhat = ln_pool.tile([128, E], dt.bfloat16, tag="ln_xhat")
                    nc.vector.tensor_scalar(out=xhat[:], in0=src_tile[:, c, :],
                                            scalar1=mvs[:, c, 0:1],
                                            scalar2=rstd[:, c:c + 1],
                                            op0=ALU.subtract, op1=ALU.mult)
                    col = dst_col0 + c * 128
                    if identity_ln:
                        for half in range(2):
                            pt = ps_pool.tile([128, 3, 128], dt.bfloat16,
                                              space="PSUM", tag="ln_tp3")
                            for j in range(3):
                                ec = half * 3 + j
                                nc.tensor.transpose(
                                    pt[:, j, :], xhat[:, ec * 128:(ec + 1) * 128],
                                    ident[:])
                            nc.scalar.activation(
                                out=dstT[:, half * 3:(half + 1) * 3, col:col + 128],
                                in_=pt[:], func=AF.Identity, bias=0.0, scale=1.0)
                    else:
                        for ec in range(EC):
                            pt = ps_pool.tile([128, 128], dt.bfloat16,
                                              space="PSUM", tag="ln_tp")
                            nc.tensor.transpose(
                                pt[:], xhat[:, ec * 128:(ec + 1) * 128], ident[:])
                            nc.scalar.activation(
                                out=dstT[:, ec, col:col + 128],
                                in_=pt[:], func=AF.Identity,
                                bias=lnb[:, ec:ec + 1], scale=lnw[:, ec:ec + 1])

            # ================= phase A: gather + LN0 -> snT =================
            snT = pool_snT.tile([128, EC, LK], dt.bfloat16)
            with tc.tile_pool(name="phA", bufs=2) as phA, \
                 tc.tile_pool(name="phA_ps", bufs=2, space="PSUM") as phA_ps:
                mxall = phA.tile([128, KC, E], dt.float32, tag="mxall", bufs=1)
                for kc in range(KC):
                    nc.gpsimd.indirect_dma_start(
                        out=mxall[:, kc, :], out_offset=None, in_=embcat,
                        in_offset=bass.IndirectOffsetOnAxis(
                            ap=gidx_t[:, kc:kc + 1], axis=0))
                layernorm_T(mxall[:], KC, ln0w_t, ln0b_t, ln_ident[0],
                            snT, phA, phA_ps)

            # ================= phase B: K^T and V_aug =================
            KT = pool_kv.tile([128, EC, LK], dt.bfloat16)
            Vaug = pool_kv.tile([128, KC, H * (DH + 1)], dt.bfloat16)
            nc.vector.memset(
                Vaug[:].rearrange("p c (h x) -> p c h x", x=DH + 1)[:, :, :, DH:DH + 1],
                1.0)
            with tc.tile_pool(name="phB_ps", bufs=3, space="PSUM") as phB_ps:
                for dc in range(EC):
                    for ks in range(2):
                        pk = phB_ps.tile([128, 512], dt.float32, space="PSUM",
                                         tag="pk")
                        for ec in range(EC):
                            nc.tensor.matmul(
                                pk[:], kwT_t[:, ec, dc * 128:(dc + 1) * 128],
                                snT[:, ec, ks * 512:(ks + 1) * 512],
                                start=(ec == 0), stop=(ec == EC - 1))
                        nc.vector.tensor_scalar_add(
                            KT[:, dc, ks * 512:(ks + 1) * 512], pk[:],
                            kb_t[:, dc:dc + 1])
                for kc in range(KC):
                    for hf in range(2):
                        pv = phB_ps.tile([128, 384], dt.float32, space="PSUM",
                                         tag="pv")
                        for ec in range(EC):
                            nc.tensor.matmul(
                                pv[:], snT[:, ec, kc * 128:(kc + 1) * 128],
                                vwT_t[:, ec, hf * 384:(hf + 1) * 384],
                                start=(ec == 0), stop=(ec == EC - 1))
                        dstv = Vaug[:, kc, :].rearrange(
                            "p (h x) -> p h x", x=DH + 1)[:, hf * 6:(hf + 1) * 6, 0:DH]
                        nc.vector.scalar_tensor_tensor(
                            out=dstv, in0=pv[:].rearrange("p (h x) -> p h x", x=DH),
                            scalar=1.0,
                            in1=vb_t[:, hf * 384:(hf + 1) * 384].rearrange(
                                "p (h x) -> p h x", x=DH),
                            op0=ALU.mult, op1=ALU.add)
            if "snT" in dbg:
                nc.sync.dma_start(dbg["snT"], snT[:])
            if "KT" in dbg:
                nc.sync.dma_start(dbg["KT"], KT[:])
            pool_snT.release()

            # ================= phase C: x load, LN1 -> xnT, Q^T =============
            QT = pool_qt.tile([128, EC, LQ], dt.bfloat16)
            with tc.tile_pool(name="phC", bufs=2) as phC, \
                 tc.tile_pool(name="phC_ps", bufs=3, space="PSUM") as phC_ps:
                xc = phC.tile([128, QC, E], dt.float32, tag="xc", bufs=1)
                nc.sync.dma_start(xc[:], xh.rearrange("(c p) e -> p c e", p=128))
                xnT = phC.tile([128, EC, LQ], dt.bfloat16, tag="xnT", bufs=1)
                layernorm_T(xc[:], QC, ln1w_t, ln1b_t, ln_ident[1], xnT,
                            phC, phC_ps)
                for dc in range(EC):
                    pq = phC_ps.tile([128, 512], dt.float32, space="PSUM", tag="pq")
                    for ec in range(EC):
                        nc.tensor.matmul(
                            pq[:], qwT_t[:, ec, dc * 128:(dc + 1) * 128],
                            xnT[:, ec, :],
                            start=(ec == 0), stop=(ec == EC - 1))
                    nc.vector.tensor_scalar_add(QT[:, dc, :], pq[:],
                                                qb_t[:, dc:dc + 1])
            if "QT" in dbg:
                nc.sync.dma_start(dbg["QT"], QT[:])
            pool_qkv.release()

            # ================= mask =================
            need_mask = [any(plan[qp][kc] == 1 for qp in range(NQP))
                         for kc in range(KC)]
            mask = pool_mask.tile([128, KC, LQ], dt.bfloat16)
            nonneg = singles.tile([128, KC], dt.float32)
            nc.vector.tensor_scalar(out=nonneg[:], in0=modage_t[:], scalar1=0.0,
                                    scalar2=None, op0=ALU.is_ge)
            for kc in range(KC):
                if not need_mask[kc]:
                    continue
                nc.vector.tensor_scalar(out=mask[:, kc, :], in0=age_bc[:],
                                        scalar1=modage_t[:, kc:kc + 1],
                                        scalar2=None, op0=ALU.is_ge)
                nc.vector.tensor_scalar_mul(mask[:, kc, :], mask[:, kc, :],
                                            nonneg[:, kc:kc + 1])

            # ================= phase D: attention =================
            # Per-kc schedule shared by every head: merge the two query
            # windows into one wide op when both are live and have matching
            # accumulation state; per-window ops otherwise.
            last_live = [max((kc for kc in range(KC) if plan[qp][kc] != 0),
                             default=-1) for qp in range(NQP)]
            sched = []  # (kc, c0, c1, start, stop, mask_slices)
            seen = [False] * NQP
            for kc in range(KC):
                lv = [qp for qp in range(NQP) if plan[qp][kc] != 0]
                if not lv:
                    continue
                if len(lv) == 2 and seen[0] == seen[1]:
                    groups = [(0, 2 * QW, lv)]
                else:
                    groups = [(qp * QW, (qp + 1) * QW, [qp]) for qp in lv]
                for c0, c1, qps in groups:
                    msl = []
                    if all(plan[qp][kc] == 1 for qp in qps):
                        msl = [(c0, c1)]
                    else:
                        msl = [(qp * QW, (qp + 1) * QW) for qp in qps
                               if plan[qp][kc] == 1]
                    sched.append((kc, c0, c1, not seen[qps[0]],
                                  all(kc == last_live[qp] for qp in qps), msl))
                for qp in lv:
                    seen[qp] = True

            Yall = pool_yall.tile([128, EC, LQ], dt.bfloat16)
            with tc.tile_pool(name="phD", bufs=10) as phD, \
                 tc.tile_pool(name="phD_ps", bufs=2, space="PSUM") as phD_ps, \
                 tc.tile_pool(name="phD_psy", bufs=2, space="PSUM") as phD_psy:
                for hp in range(H // 2):
                    dc = hp
                    # both heads of the pair share one 2-bank psum + pt tile
                    py = phD_psy.tile([128, 2, LQ], dt.float32, space="PSUM",
                                      tag="py")
                    pts = []
                    for kc, c0, c1, st, sp, msl in sched:
                        w = c1 - c0
                        ps = phD_ps.tile([128, 2, 2 * QW], dt.float32,
                                         space="PSUM", tag="ps")
                        for hi in range(2):
                            nc.tensor.matmul(
                                ps[:, hi, 0:w],
                                KT[hi * DH:(hi + 1) * DH, dc,
                                   kc * 128:(kc + 1) * 128],
                                QT[hi * DH:(hi + 1) * DH, dc, c0:c1],
                                start=True, stop=True, skip_group_check=True)
                        pt = phD.tile([128, 2, 2 * QW], dt.bfloat16, tag="pt")
                        nc.scalar.activation(out=pt[:, :, 0:w], in_=ps[:, :, 0:w],
                                             func=AF.Exp, bias=0.0, scale=1.0)
                        for m0, m1 in msl:
                            mk = mask[:, kc, m0:m1]
                            mk2 = bass.AP(tensor=mk.tensor, offset=mk.offset,
                                          ap=[mk.ap[0], [0, 2], mk.ap[1]])
                            nc.vector.tensor_tensor(
                                out=pt[:, :, m0 - c0:m1 - c0],
                                in0=pt[:, :, m0 - c0:m1 - c0],
                                in1=mk2, op=ALU.mult)
                        pts.append(pt)
                    for hi in range(2):
                        h = 2 * hp + hi
                        for (kc, c0, c1, st, sp, msl), pt in zip(sched, pts):
                            nc.tensor.matmul(
                                py[0:DH + 1, hi, c0:c1],
                                Vaug[:, kc, h * (DH + 1):(h + 1) * (DH + 1)],
                                pt[:, hi, 0:c1 - c0],
                                start=st, stop=sp, skip_group_check=True)
                    for hi in range(2):
                        rec = phD.tile([1, LQ], dt.float32, tag="rec")
                        nc.vector.tensor_scalar_add(rec[:], py[DH:DH + 1, hi, :],
                                                    1e-30)
                        nc.vector.reciprocal_approx_fast(out=rec[:], in_=rec[:])
                        recb = phD.tile([DH, LQ], dt.float32, tag="recb")
                        nc.gpsimd.partition_broadcast(recb[:], rec[:])
                        nc.vector.tensor_mul(
                            out=Yall[hi * DH:(hi + 1) * DH, dc, :],
                            in0=py[0:DH, hi, :], in1=recb[:])
            if "Yall" in dbg:
                nc.sync.dma_start(dbg["Yall"], Yall[:])
            pool_mask.release()
            pool_qt.release()
            pool_kv.release()

            # ============ phase E: c-proj + residual, LN2 -> h1nT ============
            x2 = pool_x2.tile([128, QC, E], dt.float32)
            with tc.tile_pool(name="phE", bufs=1) as phE, \
                 tc.tile_pool(name="phE_ps", bufs=4, space="PSUM") as phE_ps:
                xe = phE.tile([128, QC, E], dt.float32, tag="xe")
                nc.sync.dma_start(xe[:], xh.rearrange("(c p) e -> p c e", p=128))
                for qc in range(QC):
                    for hf in range(2):
                        pc = phE_ps.tile([128, 384], dt.float32, space="PSUM",
                                         tag="pc")
                        for ec in range(EC):
                            nc.tensor.matmul(
                                pc[:], Yall[:, ec, qc * 128:(qc + 1) * 128],
                                cwT_t[:, ec, hf * 384:(hf + 1) * 384],
                                start=(ec == 0), stop=(ec == EC - 1))
                        sl = slice(hf * 384, (hf + 1) * 384)
                        nc.vector.scalar_tensor_tensor(
                            out=x2[:, qc, sl], in0=pc[:], scalar=1.0,
                            in1=cb_t[:, sl], op0=ALU.mult, op1=ALU.add)
                    nc.vector.tensor_add(out=x2[:, qc, :], in0=x2[:, qc, :],
                                         in1=xe[:, qc, :])
            if "x2" in dbg:
                nc.sync.dma_start(dbg["x2"], x2[:])

            pool_h1 = tc.alloc_tile_pool(name="pool_h1", bufs=1)
            h1nT = pool_h1.tile([128, EC, LQ], dt.bfloat16)
            with tc.tile_pool(name="phE2", bufs=2) as phE2, \
                 tc.tile_pool(name="phE2_ps", bufs=3, space="PSUM") as phE2_ps:
                layernorm_T(x2[:], QC, ln2w_t, ln2b_t, ln_ident[2], h1nT,
                            phE2, phE2_ps)

            # ================= phase F: MLP =================
            pool_hT = tc.alloc_tile_pool(name="pool_hT", bufs=1)
            hT = pool_hT.tile([128, FC, LQ], dt.bfloat16)
            with tc.tile_pool(name="phF", bufs=3) as phF, \
                 tc.tile_pool(name="phF_ps", bufs=2, space="PSUM") as phF_ps:
                for f in range(FC):
                    fw = phF.tile([128, EC, 128], dt.bfloat16, tag="fw")
                    nc.sync.dma_start(fw[:], fcw_img[f]
                                      .rearrange("p (c x) -> p c x", c=EC))
                    ph = phF_ps.tile([128, LQ], dt.float32, space="PSUM", tag="ph")
                    for ec in range(EC):
                        nc.tensor.matmul(ph[:], fw[:, ec, :], h1nT[:, ec, :],
                                         start=(ec == 0), stop=(ec == EC - 1))
                    nc.scalar.activation(out=hT[:, f, :], in_=ph[:], func=AF.Gelu,
                                         bias=fcb_t[:, f:f + 1], scale=1.0)

            with tc.tile_pool(name="phG", bufs=3) as phG, \
                 tc.tile_pool(name="phG_ps", bufs=8, space="PSUM") as phG_ps, \
                 tc.tile_pool(name="phG_out", bufs=2) as phG_out:
                pps = [phG_ps.tile([128, 384], dt.float32, space="PSUM", tag="pp",
                                   name=f"pp{i}")
                       for i in range(2 * QC)]
                for f in range(FC):
                    pw = phG.tile([128, E], dt.bfloat16, tag="pw")
                    nc.sync.dma_start(pw[:], pwT[f * 128:(f + 1) * 128, :])
                    for qc in range(QC):
                        for hf in range(2):
                            nc.tensor.matmul(
                                pps[qc * 2 + hf][:],
                                hT[:, f, qc * 128:(qc + 1) * 128],
                                pw[:, hf * 384:(hf + 1) * 384],
                                start=(f == 0), stop=(f == FC - 1),
                                skip_group_check=True)
                for qc in range(QC):
                    ot = phG_out.tile([128, E], dt.float32, tag="ot")
                    for hf in range(2):
                        sl = slice(hf * 384, (hf + 1) * 384)
                        nc.vector.scalar_tensor_tensor(
                            out=ot[:, sl], in0=pps[qc * 2 + hf][:], scalar=1.0,
                            in1=pb_t[:, sl], op0=ALU.mult, op1=ALU.add)
                    nc.vector.tensor_add(out=ot[:], in0=ot[:], in1=x2[:, qc, :])
                    nc.sync.dma_start(
                        out.rearrange("(c p) e -> p c e", p=128)[:, qc, :], ot[:])

            pool_hT.release()
            pool_h1.release()
            pool_x2.release()
            pool_yall.release()
            pool_cw.release()
            singles.release()

    nc.compile()
    return nc


def _to_img(wT):
    """[E, D] (e-major) -> SBUF image [128, EC*D]: img[p, c*D+d] = wT[c*128+p, d]."""
    Ei, D = wT.shape
    return np.ascontiguousarray(
        wT.reshape(Ei // 128, 128, D).transpose(1, 0, 2).reshape(128, -1))


def _host_prep(x, age, mod_idx, mod_age, mod2_emb, mod3_emb,
               ln0_w, ln0_b, ln1_w, ln1_b, ln2_w, ln2_b,
               q_w, q_b, k_w, k_b, v_w, v_b, c_w, c_b,
               fc_w, fc_b, proj_w, proj_b):
    f32 = np.float32
    x = np.asarray(x, f32); age = np.asarray(age, f32)
    mod_idx = np.asarray(mod_idx); mod_age = np.asarray(mod_age, f32)
    mod2_emb = np.asarray(mod2_emb, f32); mod3_emb = np.asarray(mod3_emb, f32)

    scale = np.float32(DH) ** -0.5
    qw_img = _to_img(np.asarray(q_w, f32).T * scale).astype(BF)
    kw_img = _to_img(np.asarray(k_w, f32).T).astype(BF)
    vw_img = _to_img(np.asarray(v_w, f32).T).astype(BF)
    cw_img = _to_img(np.asarray(c_w, f32).T).astype(BF)
    fcw_img = np.ascontiguousarray(
        np.asarray(fc_w, f32).T.reshape(EC, 128, FC, 128)
        .transpose(2, 1, 0, 3).reshape(FC, 128, E)).astype(BF)
    pwT = np.ascontiguousarray(np.asarray(proj_w, f32).T).astype(BF)
    qb2 = np.asarray(q_b, f32) * scale

    lnp = [np.asarray(a, f32) for a in
           (ln0_w, ln0_b, ln1_w, ln1_b, ln2_w, ln2_b)]
    ln_ident = tuple(
        bool(np.all(lnp[2 * i] == 1.0) and np.all(lnp[2 * i + 1] == 0.0))
        for i in range(3))

    shared = dict(
        qw_img=qw_img, kw_img=kw_img, vw_img=vw_img, cw_img=cw_img,
        fcw_img=fcw_img, pwT=pwT,
        qb=qb2, kb=np.asarray(k_b, f32), vb=np.asarray(v_b, f32),
        cb=np.asarray(c_b, f32), fcb=np.asarray(fc_b, f32),
        pb=np.asarray(proj_b, f32),
        ln0w=lnp[0], ln0b=lnp[1], ln1w=lnp[2], ln1b=lnp[3],
        ln2w=lnp[4], ln2b=lnp[5],
    )

    qrows = {0: np.array([t * 128 + i for t in QTILES_G0 for i in range(128)]),
             1: np.array([t * 128 + i for t in QTILES_G1 for i in range(128)])}

    in_maps = []
    plan_counts = np.zeros((NQP, KC, 3), dtype=np.int64)
    for core in range(N_CORES):
        b, g = core // 2, core % 2
        rows = qrows[g]
        order = np.argsort(mod_age[b], kind="stable")
        s_idx = np.asarray(mod_idx[b])[order]
        m2 = s_idx == 2
        m3 = s_idx == 3
        occ2 = np.clip(np.cumsum(m2) - 1, 0, HALF - 1)
        occ3 = np.clip(np.cumsum(m3) - 1, 0, HALF - 1)
        gi = np.full(LK, 2 * HALF, dtype=np.int32)
        gi[m2] = occ2[m2]
        gi[m3] = HALF + occ3[m3]
        embcat = np.concatenate([
            mod2_emb[b * HALF:(b + 1) * HALF],
            mod3_emb[b * HALF:(b + 1) * HALF],
            np.zeros((1, E), f32)], axis=0)
        agec = age[b][rows]
        mk = (agec[None, :] >= mod_age[b][:, None]) & (mod_age[b] >= 0.0)[:, None]
        for qp in range(NQP):
            sub = mk[:, qp * QW:(qp + 1) * QW]
            for kc in range(KC):
                blk = sub[kc * 128:(kc + 1) * 128]
                if not blk.any():
                    plan_counts[qp, kc, 0] += 1
                elif blk.all():
                    plan_counts[qp, kc, 2] += 1
                else:
                    plan_counts[qp, kc, 1] += 1
        in_maps.append(dict(
            xh=np.ascontiguousarray(x[b][rows]),
            embcat=embcat, gidx=gi,
            ageh=np.ascontiguousarray(agec),
            modage=np.ascontiguousarray(mod_age[b]),
            **shared))

    plan = []
    for qp in range(NQP):
        row = []
        for kc in range(KC):
            z, p, fl = plan_counts[qp, kc]
            if z == N_CORES:
                row.append(0)
            elif fl == N_CORES:
                row.append(2)
            else:
                row.append(1)
        if all(a == 0 for a in row):
            row[0] = 1
        plan.append(tuple(row))
    return in_maps, (tuple(plan), ln_ident), qrows


def _run(inputs, trace):
    in_maps, key, qrows = _host_prep(**inputs)
    if key not in _prog_cache:
        _prog_cache[key] = _build_program(*key)
    nc = _prog_cache[key]
    res = run_bass_kernel_spmd(nc, in_maps, core_ids=list(range(N_CORES)),
                               trace=trace)
    out = np.empty((B, 2 * LQ, E), dtype=np.float32)
    for core in range(N_CORES):
        b, g = core // 2, core % 2
        out[b, qrows[g]] = res.results[core]["out"]
    return out, res


def kernel(**inputs):
    return _run(inputs, trace=False)[0]


def run_traced(**inputs):
    return _run(inputs, trace=True)


# revision 39
# speedup vs baseline: 1.4124x; 1.0062x over previous
"""Trainium2 Bass kernel for a cross-attention transformer block.

Sharding: 8 cores = 4 batch rows x 2 query-groups of 512.
Query groups are quarter-interleaved (q-tiles {0,1,4,5} vs {2,3,6,7} of 128
queries) so the SPMD-shared attention chunk plan skips similar amounts of
masked work on every core.

On-device: modality-embedding gather (indirect DMA, host-computed row
indices), layernorms (bn_stats + PE-transpose into [E, tokens] layout with
LN scale/bias fused into the ACT psum->sbuf copy), QKV / attention / MLP
matmuls in bf16 with fp32 accumulation, masked softmax as exp(S) * mask
with a ones-row appended to V producing denominators for free, fused Gelu.
Host computes gather indices (argsort/cumsum), pre-tiles weights into
SBUF-image layouts, and slices/reassembles per-core tensors.

The attention "plan" classifies each (query-window, key-chunk) block of the
time mask as all-zero (skip S/exp/PV entirely), all-one (no mask multiply),
or partial (multiply by the mask tile). It is computed from the actual
input mask (union over cores, since SPMD shares one program), so it is
exact for arbitrary inputs; sorted ages just make it effective.
"""

import numpy as np
import ml_dtypes

import concourse.bass as bass
import concourse.tile as tile
from concourse import bacc, mybir
from concourse.bass_utils import run_bass_kernel_spmd
from concourse.masks import make_identity

dt = mybir.dt
AF = mybir.ActivationFunctionType
ALU = mybir.AluOpType

B = 4
LQ = 512          # queries per core
LK = 1024         # side (key) sequence length
E = 768
H = 12
DH = 64           # head dim
F = 3072
HALF = 512        # occurrences of each modality per row
EC = E // 128     # 6
FC = F // 128     # 24
KC = LK // 128    # 8
QC = LQ // 128    # 4
NQP = 2           # query windows of 256 (plan granularity)
QW = LQ // NQP    # 256
LN_EPS = 1e-5
N_CORES = 8
BF = ml_dtypes.bfloat16

QTILES_G0 = [0, 1, 4, 5]
QTILES_G1 = [2, 3, 6, 7]

_prog_cache = {}

DEBUG_NAMES = ()


def _build_program(plan, ln_ident):
    """plan: NQP x KC ints (0=skip,1=mask,2=full). ln_ident: 3 bools - LN
    weight==1 & bias==0, enabling batched plain transpose-copies."""
    nc = bacc.Bacc("TRN2", target_bir_lowering=False, debug=False,
                   num_devices=N_CORES)

    def din(name, shape, dty=dt.float32):
        return nc.dram_tensor(name, shape, dty, kind="ExternalInput").ap()

    xh = din("xh", [LQ, E])
    embcat = din("embcat", [2 * HALF + 1, E])
    gidx = din("gidx", [LK], dt.int32)
    ageh = din("ageh", [LQ])
    modage = din("modage", [LK])
    qw_img = din("qw_img", [128, EC * E], dt.bfloat16)
    kw_img = din("kw_img", [128, EC * E], dt.bfloat16)
    vw_img = din("vw_img", [128, EC * E], dt.bfloat16)
    cw_img = din("cw_img", [128, EC * E], dt.bfloat16)
    fcw_img = din("fcw_img", [FC, 128, E], dt.bfloat16)
    pwT = din("pwT", [F, E], dt.bfloat16)
    qb = din("qb", [E])
    kb = din("kb", [E])
    vb = din("vb", [E])
    cb = din("cb", [E])
    fcb = din("fcb", [F])
    pb = din("pb", [E])
    ln0w = din("ln0w", [E]); ln0b = din("ln0b", [E])
    ln1w = din("ln1w", [E]); ln1b = din("ln1b", [E])
    ln2w = din("ln2w", [E]); ln2b = din("ln2b", [E])

    out = nc.dram_tensor("out", [LQ, E], dt.float32, kind="ExternalOutput").ap()
    dbg = {}
    def dout(name, shape, dty=dt.bfloat16):
        dbg[name] = nc.dram_tensor("dbg_" + name, shape, dty,
                                   kind="ExternalOutput").ap()
    for nm in DEBUG_NAMES:
        if nm == "x2": dout(nm, [128, QC, E], dt.float32)
        if nm == "snT": dout(nm, [128, EC, LK])
        if nm == "KT": dout(nm, [128, EC, LK])
        if nm == "QT": dout(nm, [128, EC, LQ])
        if nm == "Yall": dout(nm, [128, EC, LQ])

    with tile.TileContext(nc) as tc:
        if True:
            # LIFO pool stack: opened in reverse order of release.
            singles = tc.alloc_tile_pool(name="singles", bufs=1)
            pool_cw = tc.alloc_tile_pool(name="pool_cw", bufs=1)
            pool_yall = tc.alloc_tile_pool(name="pool_yall", bufs=1)
            pool_x2 = tc.alloc_tile_pool(name="pool_x2", bufs=1)
            pool_kv = tc.alloc_tile_pool(name="pool_kv", bufs=1)
            pool_qt = tc.alloc_tile_pool(name="pool_qt", bufs=1)
            pool_mask = tc.alloc_tile_pool(name="pool_mask", bufs=1)
            pool_qkv = tc.alloc_tile_pool(name="pool_qkv", bufs=1)
            pool_snT = tc.alloc_tile_pool(name="pool_snT", bufs=1)

            # ---------------- constants / small params ----------------
            ident = singles.tile([128, 128], dt.bfloat16)
            make_identity(nc, ident[:])
            eps_t = singles.tile([128, 1], dt.float32)
            nc.vector.memset(eps_t[:], LN_EPS)

            def col6(name, ap):  # [E] -> [128, EC] per-chunk columns
                t = singles.tile([128, EC], dt.float32, tag=name)
                nc.sync.dma_start(t[:], ap.rearrange("(c p) -> p c", p=128))
                return t

            qb_t = col6("qb", qb); kb_t = col6("kb", kb)
            ln0w_t = col6("ln0w", ln0w); ln0b_t = col6("ln0b", ln0b)
            ln1w_t = col6("ln1w", ln1w); ln1b_t = col6("ln1b", ln1b)
            ln2w_t = col6("ln2w", ln2w); ln2b_t = col6("ln2b", ln2b)
            fcb_t = singles.tile([128, FC], dt.float32)
            nc.sync.dma_start(fcb_t[:], fcb.rearrange("(c p) -> p c", p=128))

            def bcast768(name, ap):  # [E] -> [128, E] partition-broadcast
                t = singles.tile([128, E], dt.float32, tag=name)
                src = bass.AP(tensor=ap.tensor, offset=ap.offset,
                              ap=[[0, 128]] + ap.ap)
                nc.sync.dma_start(t[:], src)
                return t

            vb_t = bcast768("vb", vb)
            cb_t = bcast768("cb", cb)
            pb_t = bcast768("pb", pb)

            age_bc = singles.tile([128, LQ], dt.float32)
            nc.sync.dma_start(age_bc[:], bass.AP(
                tensor=ageh.tensor, offset=ageh.offset, ap=[[0, 128]] + ageh.ap))
            modage_t = singles.tile([128, KC], dt.float32)
            nc.sync.dma_start(modage_t[:], modage.rearrange("(c p) -> p c", p=128))
            gidx_t = singles.tile([128, KC], dt.int32)
            nc.sync.dma_start(gidx_t[:], gidx.rearrange("(c p) -> p c", p=128))

            def wload(pool, name, img):
                t = pool.tile([128, EC, E], dt.bfloat16, tag=name)
                nc.sync.dma_start(t[:], img.rearrange("p (c d) -> p c d", d=E))
                return t

            qwT_t = wload(pool_qkv, "qwT", qw_img)
            kwT_t = wload(pool_qkv, "kwT", kw_img)
            vwT_t = wload(pool_qkv, "vwT", vw_img)
            cwT_t = wload(pool_cw, "cwT", cw_img)

            # ---------------- layernorm -> transposed bf16 ----------------
            def layernorm_T(src_tile, n_chunks, lnw, lnb, identity_ln,
                            dstT, ln_pool, ps_pool, dst_col0=0,
                            pipelined=False):
                """src_tile: [128, n_chunks, E] f32 tokens-on-partitions.
                dstT: [128, EC, cols] bf16; writes cols [dst_col0,
                dst_col0+128*n_chunks), LN w/b fused into the copy."""
                mvs = ln_pool.tile([128, n_chunks, 2], dt.float32, tag="ln_mvs")
                rstd = ln_pool.tile([128, n_chunks], dt.float32, tag="ln_rstd")
                bs = 1 if pipelined else n_chunks
                for c0 in range(0, n_chunks, bs):
                    for c in range(c0, c0 + bs):
                        stats = ln_pool.tile([128, 3, 6], dt.float32,
                                             tag="ln_stats")
                        for sg in range(3):
                            nc.vector.bn_stats(out=stats[:, sg, :],
                                               in_=src_tile[:, c, sg * 256:(sg + 1) * 256])
                        nc.vector.bn_aggr(out=mvs[:, c, :], in_=stats[:])
                    nc.scalar.activation(out=rstd[:, c0:c0 + bs],
                                         in_=mvs[:, c0:c0 + bs, 1], func=AF.Sqrt,
                                         bias=eps_t[:], scale=1.0)
                    nc.vector.reciprocal_approx_fast(out=rstd[:, c0:c0 + bs],
                                                     in_=rstd[:, c0:c0 + bs])
                for c in range(n_chunks):
                    xhat = ln_pool.tile([128, E], dt.bfloat16, tag="ln_xhat")
                    nc.vector.tensor_scalar(out=xhat[:], in0=src_tile[:, c, :],
                                            scalar1=mvs[:, c, 0:1],
                                            scalar2=rstd[:, c:c + 1],
                                            op0=ALU.subtract, op1=ALU.mult)
                    col = dst_col0 + c * 128
                    if identity_ln:
                        for half in range(2):
                            pt = ps_pool.tile([128, 3, 128], dt.bfloat16,
                                              space="PSUM", tag="ln_tp3")
                            for j in range(3):
                                ec = half * 3 + j
                                nc.tensor.transpose(
                                    pt[:, j, :], xhat[:, ec * 128:(ec + 1) * 128],
                                    ident[:])
                            nc.scalar.activation(
                                out=dstT[:, half * 3:(half + 1) * 3, col:col + 128],
                                in_=pt[:], func=AF.Identity, bias=0.0, scale=1.0)
                    else:
                        for ec in range(EC):
                            pt = ps_pool.tile([128, 128], dt.bfloat16,
                                              space="PSUM", tag="ln_tp")
                            nc.tensor.transpose(
                                pt[:], xhat[:, ec * 128:(ec + 1) * 128], ident[:])
                            nc.scalar.activation(
                                out=dstT[:, ec, col:col + 128],
                                in_=pt[:], func=AF.Identity,
                                bias=lnb[:, ec:ec + 1], scale=lnw[:, ec:ec + 1])

            # ================= phase A: gather + LN0 -> snT =================
            # ================= phase C: x load, LN1 -> xnT, Q^T =============
            QT = pool_qt.tile([128, EC, LQ], dt.bfloat16)
            with tc.tile_pool(name="phC", bufs=2) as phC, \
                 tc.tile_pool(name="phC_ps", bufs=3, space="PSUM") as phC_ps:
                xc = phC.tile([128, QC, E], dt.float32, tag="xc", bufs=1)
                nc.sync.dma_start(xc[:], xh.rearrange("(c p) e -> p c e", p=128))
                xnT = phC.tile([128, EC, LQ], dt.bfloat16, tag="xnT", bufs=1)
                layernorm_T(xc[:], QC, ln1w_t, ln1b_t, ln_ident[1], xnT,
                            phC, phC_ps, pipelined=True)
                for dc in range(EC):
                    pq = phC_ps.tile([128, 512], dt.float32, space="PSUM", tag="pq")
                    for ec in range(EC):
                        nc.tensor.matmul(
                            pq[:], qwT_t[:, ec, dc * 128:(dc + 1) * 128],
                            xnT[:, ec, :],
                            start=(ec == 0), stop=(ec == EC - 1))
                    nc.vector.tensor_scalar_add(QT[:, dc, :], pq[:],
                                                qb_t[:, dc:dc + 1])
            if "QT" in dbg:
                nc.sync.dma_start(dbg["QT"], QT[:])

            snT = pool_snT.tile([128, EC, LK], dt.bfloat16)
            with tc.tile_pool(name="phA", bufs=2) as phA, \
                 tc.tile_pool(name="phA_ps", bufs=2, space="PSUM") as phA_ps:
                mxall = phA.tile([128, KC, E], dt.float32, tag="mxall", bufs=1)
                for kc in range(KC):
                    nc.gpsimd.indirect_dma_start(
                        out=mxall[:, kc, :], out_offset=None, in_=embcat,
                        in_offset=bass.IndirectOffsetOnAxis(
                            ap=gidx_t[:, kc:kc + 1], axis=0))
                layernorm_T(mxall[:], KC, ln0w_t, ln0b_t, ln_ident[0],
                            snT, phA, phA_ps, pipelined=True)

            # ================= phase B: K^T and V_aug =================
            KT = pool_kv.tile([128, EC, LK], dt.bfloat16)
            Vaug = pool_kv.tile([128, KC, H * (DH + 1)], dt.bfloat16)
            nc.vector.memset(
                Vaug[:].rearrange("p c (h x) -> p c h x", x=DH + 1)[:, :, :, DH:DH + 1],
                1.0)
            with tc.tile_pool(name="phB_ps", bufs=3, space="PSUM") as phB_ps:
                for dc in range(EC):
                    for ks in range(2):
                        pk = phB_ps.tile([128, 512], dt.float32, space="PSUM",
                                         tag="pk")
                        for ec in range(EC):
                            nc.tensor.matmul(
                                pk[:], kwT_t[:, ec, dc * 128:(dc + 1) * 128],
                                snT[:, ec, ks * 512:(ks + 1) * 512],
                                start=(ec == 0), stop=(ec == EC - 1))
                        nc.vector.tensor_scalar_add(
                            KT[:, dc, ks * 512:(ks + 1) * 512], pk[:],
                            kb_t[:, dc:dc + 1])
                for kc in range(KC):
                    for hf in range(2):
                        pv = phB_ps.tile([128, 384], dt.float32, space="PSUM",
                                         tag="pv")
                        for ec in range(EC):
                            nc.tensor.matmul(
                                pv[:], snT[:, ec, kc * 128:(kc + 1) * 128],
                                vwT_t[:, ec, hf * 384:(hf + 1) * 384],
                                start=(ec == 0), stop=(ec == EC - 1))
                        dstv = Vaug[:, kc, :].rearrange(
                            "p (h x) -> p h x", x=DH + 1)[:, hf * 6:(hf + 1) * 6, 0:DH]
                        nc.vector.scalar_tensor_tensor(
                            out=dstv, in0=pv[:].rearrange("p (h x) -> p h x", x=DH),
                            scalar=1.0,
                            in1=vb_t[:, hf * 384:(hf + 1) * 384].rearrange(
                                "p (h x) -> p h x", x=DH),
                            op0=ALU.mult, op1=ALU.add)
            if "snT" in dbg:
                nc.sync.dma_start(dbg["snT"], snT[:])
            if "KT" in dbg:
                nc.sync.dma_start(dbg["KT"], KT[:])
            pool_snT.release()
            pool_qkv.release()

            # ================= mask =================
            need_mask = [any(plan[qp][kc] == 1 for qp in range(NQP))
                         for kc in range(KC)]
            mask = pool_mask.tile([128, KC, LQ], dt.bfloat16)
            nonneg = singles.tile([128, KC], dt.float32)
            nc.vector.tensor_scalar(out=nonneg[:], in0=modage_t[:], scalar1=0.0,
                                    scalar2=None, op0=ALU.is_ge)
            for kc in range(KC):
                if not need_mask[kc]:
                    continue
                nc.vector.tensor_scalar(out=mask[:, kc, :], in0=age_bc[:],
                                        scalar1=modage_t[:, kc:kc + 1],
                                        scalar2=None, op0=ALU.is_ge)
                nc.vector.tensor_scalar_mul(mask[:, kc, :], mask[:, kc, :],
                                            nonneg[:, kc:kc + 1])

            # ================= phase D: attention =================
            # Per-kc schedule shared by every head: merge the two query
            # windows into one wide op when both are live and have matching
            # accumulation state; per-window ops otherwise.
            last_live = [max((kc for kc in range(KC) if plan[qp][kc] != 0),
                             default=-1) for qp in range(NQP)]
            sched = []  # (kc, c0, c1, start, stop, mask_slices)
            seen = [False] * NQP
            for kc in range(KC):
                lv = [qp for qp in range(NQP) if plan[qp][kc] != 0]
                if not lv:
                    continue
                if len(lv) == 2 and seen[0] == seen[1]:
                    groups = [(0, 2 * QW, lv)]
                else:
                    groups = [(qp * QW, (qp + 1) * QW, [qp]) for qp in lv]
                for c0, c1, qps in groups:
                    msl = []
                    if all(plan[qp][kc] == 1 for qp in qps):
                        msl = [(c0, c1)]
                    else:
                        msl = [(qp * QW, (qp + 1) * QW) for qp in qps
                               if plan[qp][kc] == 1]
                    sched.append((kc, c0, c1, not seen[qps[0]],
                                  all(kc == last_live[qp] for qp in qps), msl))
                for qp in lv:
                    seen[qp] = True

            Yall = pool_yall.tile([128, EC, LQ], dt.bfloat16)
            with tc.tile_pool(name="phD", bufs=10) as phD, \
                 tc.tile_pool(name="phD_ps", bufs=2, space="PSUM") as phD_ps, \
                 tc.tile_pool(name="phD_psy", bufs=2, space="PSUM") as phD_psy:
                for hp in range(H // 2):
                    dc = hp
                    # both heads of the pair share one 2-bank psum + pt tile
                    py = phD_psy.tile([128, 2, LQ], dt.float32, space="PSUM",
                                      tag="py")
                    pts = []
                    for kc, c0, c1, st, sp, msl in sched:
                        w = c1 - c0
                        ps = phD_ps.tile([128, 2, 2 * QW], dt.float32,
                                         space="PSUM", tag="ps")
                        for hi in range(2):
                            nc.tensor.matmul(
                                ps[:, hi, 0:w],
                                KT[hi * DH:(hi + 1) * DH, dc,
                                   kc * 128:(kc + 1) * 128],
                                QT[hi * DH:(hi + 1) * DH, dc, c0:c1],
                                start=True, stop=True, skip_group_check=True)
                        pt = phD.tile([128, 2, 2 * QW], dt.bfloat16, tag="pt")
                        nc.scalar.activation(out=pt[:, :, 0:w], in_=ps[:, :, 0:w],
                                             func=AF.Exp, bias=0.0, scale=1.0)
                        for m0, m1 in msl:
                            mk = mask[:, kc, m0:m1]
                            mk2 = bass.AP(tensor=mk.tensor, offset=mk.offset,
                                          ap=[mk.ap[0], [0, 2], mk.ap[1]])
                            nc.vector.tensor_tensor(
                                out=pt[:, :, m0 - c0:m1 - c0],
                                in0=pt[:, :, m0 - c0:m1 - c0],
                                in1=mk2, op=ALU.mult)
                        pts.append(pt)
                    for hi in range(2):
                        h = 2 * hp + hi
                        for (kc, c0, c1, st, sp, msl), pt in zip(sched, pts):
                            nc.tensor.matmul(
                                py[0:DH + 1, hi, c0:c1],
                                Vaug[:, kc, h * (DH + 1):(h + 1) * (DH + 1)],
                                pt[:, hi, 0:c1 - c0],
                                start=st, stop=sp, skip_group_check=True)
                    for hi in range(2):
                        rec = phD.tile([1, LQ], dt.float32, tag="rec")
                        nc.vector.tensor_scalar_add(rec[:], py[DH:DH + 1, hi, :],
                                                    1e-30)
                        nc.vector.reciprocal_approx_fast(out=rec[:], in_=rec[:])
                        recb = phD.tile([DH, LQ], dt.float32, tag="recb")
                        nc.gpsimd.partition_broadcast(recb[:], rec[:])
                        nc.vector.tensor_mul(
                            out=Yall[hi * DH:(hi + 1) * DH, dc, :],
                            in0=py[0:DH, hi, :], in1=recb[:])
            if "Yall" in dbg:
                nc.sync.dma_start(dbg["Yall"], Yall[:])
            pool_mask.release()
            pool_qt.release()
            pool_kv.release()

            # ============ phase E: c-proj + residual, LN2 -> h1nT ============
            x2 = pool_x2.tile([128, QC, E], dt.float32)
            with tc.tile_pool(name="phE", bufs=1) as phE, \
                 tc.tile_pool(name="phE_ps", bufs=4, space="PSUM") as phE_ps:
                xe = phE.tile([128, QC, E], dt.float32, tag="xe")
                nc.sync.dma_start(xe[:], xh.rearrange("(c p) e -> p c e", p=128))
                for qc in range(QC):
                    for hf in range(2):
                        pc = phE_ps.tile([128, 384], dt.float32, space="PSUM",
                                         tag="pc")
                        for ec in range(EC):
                            nc.tensor.matmul(
                                pc[:], Yall[:, ec, qc * 128:(qc + 1) * 128],
                                cwT_t[:, ec, hf * 384:(hf + 1) * 384],
                                start=(ec == 0), stop=(ec == EC - 1))
                        sl = slice(hf * 384, (hf + 1) * 384)
                        nc.vector.scalar_tensor_tensor(
                            out=x2[:, qc, sl], in0=pc[:], scalar=1.0,
                            in1=cb_t[:, sl], op0=ALU.mult, op1=ALU.add)
                    nc.vector.tensor_add(out=x2[:, qc, :], in0=x2[:, qc, :],
                                         in1=xe[:, qc, :])
            if "x2" in dbg:
                nc.sync.dma_start(dbg["x2"], x2[:])

            pool_h1 = tc.alloc_tile_pool(name="pool_h1", bufs=1)
            h1nT = pool_h1.tile([128, EC, LQ], dt.bfloat16)
            with tc.tile_pool(name="phE2", bufs=2) as phE2, \
                 tc.tile_pool(name="phE2_ps", bufs=3, space="PSUM") as phE2_ps:
                layernorm_T(x2[:], QC, ln2w_t, ln2b_t, ln_ident[2], h1nT,
                            phE2, phE2_ps)

            # ================= phase F: MLP =================
            pool_hT = tc.alloc_tile_pool(name="pool_hT", bufs=1)
            hT = pool_hT.tile([128, FC, LQ], dt.bfloat16)
            with tc.tile_pool(name="phF", bufs=3) as phF, \
                 tc.tile_pool(name="phF_ps", bufs=2, space="PSUM") as phF_ps:
                for f in range(FC):
                    fw = phF.tile([128, EC, 128], dt.bfloat16, tag="fw")
                    nc.sync.dma_start(fw[:], fcw_img[f]
                                      .rearrange("p (c x) -> p c x", c=EC))
                    ph = phF_ps.tile([128, LQ], dt.float32, space="PSUM", tag="ph")
                    for ec in range(EC):
                        nc.tensor.matmul(ph[:], fw[:, ec, :], h1nT[:, ec, :],
                                         start=(ec == 0), stop=(ec == EC - 1))
                    nc.scalar.activation(out=hT[:, f, :], in_=ph[:], func=AF.Gelu,
                                         bias=fcb_t[:, f:f + 1], scale=1.0)

            with tc.tile_pool(name="phG", bufs=3) as phG, \
                 tc.tile_pool(name="phG_ps", bufs=8, space="PSUM") as phG_ps, \
                 tc.tile_pool(name="phG_out", bufs=2) as phG_out:
                pps = [phG_ps.tile([128, 384], dt.float32, space="PSUM", tag="pp",
                                   name=f"pp{i}")
                       for i in range(2 * QC)]
                for f in range(FC):
                    pw = phG.tile([128, E], dt.bfloat16, tag="pw")
                    nc.sync.dma_start(pw[:], pwT[f * 128:(f + 1) * 128, :])
                    for qc in range(QC):
                        for hf in range(2):
                            nc.tensor.matmul(
                                pps[qc * 2 + hf][:],
                                hT[:, f, qc * 128:(qc + 1) * 128],
                                pw[:, hf * 384:(hf + 1) * 384],
                                start=(f == 0), stop=(f == FC - 1),
                                skip_group_check=True)
                for qc in range(QC):
                    ot = phG_out.tile([128, E], dt.float32, tag="ot")
                    for hf in range(2):
                        sl = slice(hf * 384, (hf + 1) * 384)
                        nc.vector.scalar_tensor_tensor(
                            out=ot[:, sl], in0=pps[qc * 2 + hf][:], scalar=1.0,
                            in1=pb_t[:, sl], op0=ALU.mult, op1=ALU.add)
                    nc.vector.tensor_add(out=ot[:], in0=ot[:], in1=x2[:, qc, :])
                    nc.sync.dma_start(
                        out.rearrange("(c p) e -> p c e", p=128)[:, qc, :], ot[:])

            pool_hT.release()
            pool_h1.release()
            pool_x2.release()
            pool_yall.release()
            pool_cw.release()
            singles.release()

    nc.compile()
    return nc


def _to_img(wT):
    """[E, D] (e-major) -> SBUF image [128, EC*D]: img[p, c*D+d] = wT[c*128+p, d]."""
    Ei, D = wT.shape
    return np.ascontiguousarray(
        wT.reshape(Ei // 128, 128, D).transpose(1, 0, 2).reshape(128, -1))


def _host_prep(x, age, mod_idx, mod_age, mod2_emb, mod3_emb,
               ln0_w, ln0_b, ln1_w, ln1_b, ln2_w, ln2_b,
               q_w, q_b, k_w, k_b, v_w, v_b, c_w, c_b,
               fc_w, fc_b, proj_w, proj_b):
    f32 = np.float32
    x = np.asarray(x, f32); age = np.asarray(age, f32)
    mod_idx = np.asarray(mod_idx); mod_age = np.asarray(mod_age, f32)
    mod2_emb = np.asarray(mod2_emb, f32); mod3_emb = np.asarray(mod3_emb, f32)

    scale = np.float32(DH) ** -0.5
    qw_img = _to_img(np.asarray(q_w, f32).T * scale).astype(BF)
    kw_img = _to_img(np.asarray(k_w, f32).T).astype(BF)
    vw_img = _to_img(np.asarray(v_w, f32).T).astype(BF)
    cw_img = _to_img(np.asarray(c_w, f32).T).astype(BF)
    fcw_img = np.ascontiguousarray(
        np.asarray(fc_w, f32).T.reshape(EC, 128, FC, 128)
        .transpose(2, 1, 0, 3).reshape(FC, 128, E)).astype(BF)
    pwT = np.ascontiguousarray(np.asarray(proj_w, f32).T).astype(BF)
    qb2 = np.asarray(q_b, f32) * scale

    lnp = [np.asarray(a, f32) for a in
           (ln0_w, ln0_b, ln1_w, ln1_b, ln2_w, ln2_b)]
    ln_ident = tuple(
        bool(np.all(lnp[2 * i] == 1.0) and np.all(lnp[2 * i + 1] == 0.0))
        for i in range(3))

    shared = dict(
        qw_img=qw_img, kw_img=kw_img, vw_img=vw_img, cw_img=cw_img,
        fcw_img=fcw_img, pwT=pwT,
        qb=qb2, kb=np.asarray(k_b, f32), vb=np.asarray(v_b, f32),
        cb=np.asarray(c_b, f32), fcb=np.asarray(fc_b, f32),
        pb=np.asarray(proj_b, f32),
        ln0w=lnp[0], ln0b=lnp[1], ln1w=lnp[2], ln1b=lnp[3],
        ln2w=lnp[4], ln2b=lnp[5],
    )

    qrows = {0: np.array([t * 128 + i for t in QTILES_G0 for i in range(128)]),
             1: np.array([t * 128 + i for t in QTILES_G1 for i in range(128)])}

    in_maps = []
    plan_counts = np.zeros((NQP, KC, 3), dtype=np.int64)
    for core in range(N_CORES):
        b, g = core // 2, core % 2
        rows = qrows[g]
        order = np.argsort(mod_age[b], kind="stable")
        s_idx = np.asarray(mod_idx[b])[order]
        m2 = s_idx == 2
        m3 = s_idx == 3
        occ2 = np.clip(np.cumsum(m2) - 1, 0, HALF - 1)
        occ3 = np.clip(np.cumsum(m3) - 1, 0, HALF - 1)
        gi = np.full(LK, 2 * HALF, dtype=np.int32)
        gi[m2] = occ2[m2]
        gi[m3] = HALF + occ3[m3]
        embcat = np.concatenate([
            mod2_emb[b * HALF:(b + 1) * HALF],
            mod3_emb[b * HALF:(b + 1) * HALF],
            np.zeros((1, E), f32)], axis=0)
        agec = age[b][rows]
        mk = (agec[None, :] >= mod_age[b][:, None]) & (mod_age[b] >= 0.0)[:, None]
        for qp in range(NQP):
            sub = mk[:, qp * QW:(qp + 1) * QW]
            for kc in range(KC):
                blk = sub[kc * 128:(kc + 1) * 128]
                if not blk.any():
                    plan_counts[qp, kc, 0] += 1
                elif blk.all():
                    plan_counts[qp, kc, 2] += 1
                else:
                    plan_counts[qp, kc, 1] += 1
        in_maps.append(dict(
            xh=np.ascontiguousarray(x[b][rows]),
            embcat=embcat, gidx=gi,
            ageh=np.ascontiguousarray(agec),
            modage=np.ascontiguousarray(mod_age[b]),
            **shared))

    plan = []
    for qp in range(NQP):
        row = []
        for kc in range(KC):
            z, p, fl = plan_counts[qp, kc]
            if z == N_CORES:
                row.append(0)
            elif fl == N_CORES:
                row.append(2)
            else:
                row.append(1)
        if all(a == 0 for a in row):
            row[0] = 1
        plan.append(tuple(row))
    return in_maps, (tuple(plan), ln_ident), qrows


def _run(inputs, trace):
    in_maps, key, qrows = _host_prep(**inputs)
    if key not in _prog_cache:
        _prog_cache[key] = _build_program(*key)
    nc = _prog_cache[key]
    res = run_bass_kernel_spmd(nc, in_maps, core_ids=list(range(N_CORES)),
                               trace=trace)
    out = np.empty((B, 2 * LQ, E), dtype=np.float32)
    for core in range(N_CORES):
        b, g = core // 2, core % 2
        out[b, qrows[g]] = res.results[core]["out"]
    return out, res


def kernel(**inputs):
    return _run(inputs, trace=False)[0]


def run_traced(**inputs):
    return _run(inputs, trace=True)


# revision 41
# speedup vs baseline: 1.4996x; 1.0617x over previous
"""Trainium2 Bass kernel for a cross-attention transformer block.

Sharding: 8 cores = 4 batch rows x 2 query-groups of 512.
Query groups are quarter-interleaved (q-tiles {0,1,4,5} vs {2,3,6,7} of 128
queries) so the SPMD-shared attention chunk plan skips similar amounts of
masked work on every core.

On-device: modality-embedding gather (indirect DMA, host-computed row
indices), layernorms (bn_stats + PE-transpose into [E, tokens] layout with
LN scale/bias fused into the ACT psum->sbuf copy), QKV / attention / MLP
matmuls in bf16 with fp32 accumulation, masked softmax as exp(S) * mask
with a ones-row appended to V producing denominators for free, fused Gelu.
Host computes gather indices (argsort/cumsum), pre-tiles weights into
SBUF-image layouts, and slices/reassembles per-core tensors.

The attention "plan" classifies each (query-window, key-chunk) block of the
time mask as all-zero (skip S/exp/PV entirely), all-one (no mask multiply),
or partial (multiply by the mask tile). It is computed from the actual
input mask (union over cores, since SPMD shares one program), so it is
exact for arbitrary inputs; sorted ages just make it effective.
"""

import numpy as np
import ml_dtypes

import concourse.bass as bass
import concourse.tile as tile
from concourse import bacc, mybir
from concourse.bass_utils import run_bass_kernel_spmd
from concourse.masks import make_identity

dt = mybir.dt
AF = mybir.ActivationFunctionType
ALU = mybir.AluOpType

B = 4
LQ = 512          # queries per core
LK = 1024         # side (key) sequence length
E = 768
H = 12
DH = 64           # head dim
F = 3072
HALF = 512        # occurrences of each modality per row
EC = E // 128     # 6
FC = F // 128     # 24
KC = LK // 128    # 8
QC = LQ // 128    # 4
NQP = 2           # query windows of 256 (plan granularity)
QW = LQ // NQP    # 256
LN_EPS = 1e-5
N_CORES = 8
BF = ml_dtypes.bfloat16

QTILES_G0 = [0, 1, 4, 5]
QTILES_G1 = [2, 3, 6, 7]

_prog_cache = {}

DEBUG_NAMES = ()


def _build_program(plan, ln_ident):
    """plan: NQP x KC ints (0=skip,1=mask,2=full). ln_ident: 3 bools - LN
    weight==1 & bias==0, enabling batched plain transpose-copies."""
    nc = bacc.Bacc("TRN2", target_bir_lowering=False, debug=False,
                   num_devices=N_CORES)

    def din(name, shape, dty=dt.float32):
        return nc.dram_tensor(name, shape, dty, kind="ExternalInput").ap()

    xh = din("xh", [LQ, E])
    embcat = din("embcat", [2 * HALF + 1, E])
    gidx = din("gidx", [LK], dt.int32)
    ageh = din("ageh", [LQ])
    modage = din("modage", [LK])
    qw_img = din("qw_img", [128, EC * E], dt.bfloat16)
    kw_img = din("kw_img", [128, EC * E], dt.bfloat16)
    vw_img = din("vw_img", [128, EC * E], dt.bfloat16)
    cw_img = din("cw_img", [128, EC * E], dt.bfloat16)
    fcw_img = din("fcw_img", [FC, 128, E], dt.bfloat16)
    pwT = din("pwT", [F, E], dt.bfloat16)
    qb = din("qb", [E])
    kb = din("kb", [E])
    vb = din("vb", [E])
    cb = din("cb", [E])
    fcb = din("fcb", [F])
    pb = din("pb", [E])
    ln0w = din("ln0w", [E]); ln0b = din("ln0b", [E])
    ln1w = din("ln1w", [E]); ln1b = din("ln1b", [E])
    ln2w = din("ln2w", [E]); ln2b = din("ln2b", [E])

    out = nc.dram_tensor("out", [LQ, E], dt.float32, kind="ExternalOutput").ap()
    dbg = {}
    def dout(name, shape, dty=dt.bfloat16):
        dbg[name] = nc.dram_tensor("dbg_" + name, shape, dty,
                                   kind="ExternalOutput").ap()
    for nm in DEBUG_NAMES:
        if nm == "x2": dout(nm, [128, QC, E], dt.float32)
        if nm == "snT": dout(nm, [128, EC, LK])
        if nm == "KT": dout(nm, [128, EC, LK])
        if nm == "QT": dout(nm, [128, EC, LQ])
        if nm == "Yall": dout(nm, [128, EC, LQ])

    with tile.TileContext(nc) as tc:
        if True:
            # LIFO pool stack: opened in reverse order of release.
            singles = tc.alloc_tile_pool(name="singles", bufs=1)
            pool_cw = tc.alloc_tile_pool(name="pool_cw", bufs=1)
            pool_yall = tc.alloc_tile_pool(name="pool_yall", bufs=1)
            pool_x2 = tc.alloc_tile_pool(name="pool_x2", bufs=1)
            pool_kv = tc.alloc_tile_pool(name="pool_kv", bufs=1)
            pool_qt = tc.alloc_tile_pool(name="pool_qt", bufs=1)
            pool_mask = tc.alloc_tile_pool(name="pool_mask", bufs=1)
            pool_qkv = tc.alloc_tile_pool(name="pool_qkv", bufs=1)
            pool_snT = tc.alloc_tile_pool(name="pool_snT", bufs=1)

            # ---------------- constants / small params ----------------
            ident = singles.tile([128, 128], dt.bfloat16)
            make_identity(nc, ident[:])
            eps_t = singles.tile([128, 1], dt.float32)
            nc.vector.memset(eps_t[:], LN_EPS)

            def col6(name, ap):  # [E] -> [128, EC] per-chunk columns
                t = singles.tile([128, EC], dt.float32, tag=name)
                nc.sync.dma_start(t[:], ap.rearrange("(c p) -> p c", p=128))
                return t

            qb_t = col6("qb", qb); kb_t = col6("kb", kb)
            ln0w_t = col6("ln0w", ln0w); ln0b_t = col6("ln0b", ln0b)
            ln1w_t = col6("ln1w", ln1w); ln1b_t = col6("ln1b", ln1b)
            ln2w_t = col6("ln2w", ln2w); ln2b_t = col6("ln2b", ln2b)
            fcb_t = singles.tile([128, FC], dt.float32)
            nc.sync.dma_start(fcb_t[:], fcb.rearrange("(c p) -> p c", p=128))

            def bcast768(name, ap):  # [E] -> [128, E] partition-broadcast
                t = singles.tile([128, E], dt.float32, tag=name)
                src = bass.AP(tensor=ap.tensor, offset=ap.offset,
                              ap=[[0, 128]] + ap.ap)
                nc.sync.dma_start(t[:], src)
                return t

            vb_t = bcast768("vb", vb)
            cb_t = bcast768("cb", cb)
            pb_t = bcast768("pb", pb)

            age_bc = singles.tile([128, LQ], dt.float32)
            nc.sync.dma_start(age_bc[:], bass.AP(
                tensor=ageh.tensor, offset=ageh.offset, ap=[[0, 128]] + ageh.ap))
            modage_t = singles.tile([128, KC], dt.float32)
            nc.sync.dma_start(modage_t[:], modage.rearrange("(c p) -> p c", p=128))
            gidx_t = singles.tile([128, KC], dt.int32)
            nc.sync.dma_start(gidx_t[:], gidx.rearrange("(c p) -> p c", p=128))

            def wload(pool, name, img):
                t = pool.tile([128, EC, E], dt.bfloat16, tag=name)
                nc.scalar.dma_start(t[:], img.rearrange("p (c d) -> p c d", d=E))
                return t

            qwT_t = wload(pool_qkv, "qwT", qw_img)
            kwT_t = wload(pool_qkv, "kwT", kw_img)
            vwT_t = wload(pool_qkv, "vwT", vw_img)
            cwT_t = wload(pool_cw, "cwT", cw_img)

            # ---------------- layernorm -> transposed bf16 ----------------
            def layernorm_T(src_tile, n_chunks, lnw, lnb, identity_ln,
                            dstT, ln_pool, ps_pool, dst_col0=0,
                            pipelined=False):
                """src_tile: [128, n_chunks, E] f32 tokens-on-partitions.
                dstT: [128, EC, cols] bf16; writes cols [dst_col0,
                dst_col0+128*n_chunks), LN w/b fused into the copy."""
                mvs = ln_pool.tile([128, n_chunks, 2], dt.float32, tag="ln_mvs")
                rstd = ln_pool.tile([128, n_chunks], dt.float32, tag="ln_rstd")
                bs = 1 if pipelined else n_chunks
                for c0 in range(0, n_chunks, bs):
                    for c in range(c0, c0 + bs):
                        stats = ln_pool.tile([128, 3, 6], dt.float32,
                                             tag="ln_stats")
                        for sg in range(3):
                            nc.vector.bn_stats(out=stats[:, sg, :],
                                               in_=src_tile[:, c, sg * 256:(sg + 1) * 256])
                        nc.vector.bn_aggr(out=mvs[:, c, :], in_=stats[:])
                    nc.scalar.activation(out=rstd[:, c0:c0 + bs],
                                         in_=mvs[:, c0:c0 + bs, 1], func=AF.Sqrt,
                                         bias=eps_t[:], scale=1.0)
                    nc.vector.reciprocal_approx_fast(out=rstd[:, c0:c0 + bs],
                                                     in_=rstd[:, c0:c0 + bs])
                for c in range(n_chunks):
                    xhat = ln_pool.tile([128, E], dt.bfloat16, tag="ln_xhat")
                    nc.vector.tensor_scalar(out=xhat[:], in0=src_tile[:, c, :],
                                            scalar1=mvs[:, c, 0:1],
                                            scalar2=rstd[:, c:c + 1],
                                            op0=ALU.subtract, op1=ALU.mult)
                    col = dst_col0 + c * 128
                    if identity_ln:
                        for half in range(2):
                            pt = ps_pool.tile([128, 3, 128], dt.bfloat16,
                                              space="PSUM", tag="ln_tp3")
                            for j in range(3):
                                ec = half * 3 + j
                                nc.tensor.transpose(
                                    pt[:, j, :], xhat[:, ec * 128:(ec + 1) * 128],
                                    ident[:])
                            nc.scalar.activation(
                                out=dstT[:, half * 3:(half + 1) * 3, col:col + 128],
                                in_=pt[:], func=AF.Identity, bias=0.0, scale=1.0)
                    else:
                        for ec in range(EC):
                            pt = ps_pool.tile([128, 128], dt.bfloat16,
                                              space="PSUM", tag="ln_tp")
                            nc.tensor.transpose(
                                pt[:], xhat[:, ec * 128:(ec + 1) * 128], ident[:])
                            nc.scalar.activation(
                                out=dstT[:, ec, col:col + 128],
                                in_=pt[:], func=AF.Identity,
                                bias=lnb[:, ec:ec + 1], scale=lnw[:, ec:ec + 1])

            # ================= phase A: gather + LN0 -> snT =================
            # ================= phase C: x load, LN1 -> xnT, Q^T =============
            QT = pool_qt.tile([128, EC, LQ], dt.bfloat16)
            with tc.tile_pool(name="phC", bufs=2) as phC, \
                 tc.tile_pool(name="phC_ps", bufs=3, space="PSUM") as phC_ps:
                xc = phC.tile([128, QC, E], dt.float32, tag="xc", bufs=1)
                nc.sync.dma_start(xc[:], xh.rearrange("(c p) e -> p c e", p=128))
                xnT = phC.tile([128, EC, LQ], dt.bfloat16, tag="xnT", bufs=1)
                layernorm_T(xc[:], QC, ln1w_t, ln1b_t, ln_ident[1], xnT,
                            phC, phC_ps, pipelined=True)
                for dc in range(EC):
                    pq = phC_ps.tile([128, 512], dt.float32, space="PSUM", tag="pq")
                    for ec in range(EC):
                        nc.tensor.matmul(
                            pq[:], qwT_t[:, ec, dc * 128:(dc + 1) * 128],
                            xnT[:, ec, :],
                            start=(ec == 0), stop=(ec == EC - 1))
                    nc.vector.tensor_scalar_add(QT[:, dc, :], pq[:],
                                                qb_t[:, dc:dc + 1])
            if "QT" in dbg:
                nc.sync.dma_start(dbg["QT"], QT[:])

            snT = pool_snT.tile([128, EC, LK], dt.bfloat16)
            with tc.tile_pool(name="phA", bufs=2) as phA, \
                 tc.tile_pool(name="phA_ps", bufs=2, space="PSUM") as phA_ps:
                mxall = phA.tile([128, KC, E], dt.float32, tag="mxall", bufs=1)
                for kc in range(KC):
                    nc.gpsimd.indirect_dma_start(
                        out=mxall[:, kc, :], out_offset=None, in_=embcat,
                        in_offset=bass.IndirectOffsetOnAxis(
                            ap=gidx_t[:, kc:kc + 1], axis=0))
                layernorm_T(mxall[:], KC, ln0w_t, ln0b_t, ln_ident[0],
                            snT, phA, phA_ps, pipelined=True)

            # ================= phase B: K^T and V_aug =================
            KT = pool_kv.tile([128, EC, LK], dt.bfloat16)
            Vaug = pool_kv.tile([128, KC, H * (DH + 1)], dt.bfloat16)
            nc.vector.memset(
                Vaug[:].rearrange("p c (h x) -> p c h x", x=DH + 1)[:, :, :, DH:DH + 1],
                1.0)
            with tc.tile_pool(name="phB_ps", bufs=3, space="PSUM") as phB_ps:
                for dc in range(EC):
                    for ks in range(2):
                        pk = phB_ps.tile([128, 512], dt.float32, space="PSUM",
                                         tag="pk")
                        for ec in range(EC):
                            nc.tensor.matmul(
                                pk[:], kwT_t[:, ec, dc * 128:(dc + 1) * 128],
                                snT[:, ec, ks * 512:(ks + 1) * 512],
                                start=(ec == 0), stop=(ec == EC - 1))
                        nc.vector.tensor_scalar_add(
                            KT[:, dc, ks * 512:(ks + 1) * 512], pk[:],
                            kb_t[:, dc:dc + 1])
                for kc in range(KC):
                    for hf in range(2):
                        pv = phB_ps.tile([128, 384], dt.float32, space="PSUM",
                                         tag="pv")
                        for ec in range(EC):
                            nc.tensor.matmul(
                                pv[:], snT[:, ec, kc * 128:(kc + 1) * 128],
                                vwT_t[:, ec, hf * 384:(hf + 1) * 384],
                                start=(ec == 0), stop=(ec == EC - 1))
                        dstv = Vaug[:, kc, :].rearrange(
                            "p (h x) -> p h x", x=DH + 1)[:, hf * 6:(hf + 1) * 6, 0:DH]
                        nc.vector.scalar_tensor_tensor(
                            out=dstv, in0=pv[:].rearrange("p (h x) -> p h x", x=DH),
                            scalar=1.0,
                            in1=vb_t[:, hf * 384:(hf + 1) * 384].rearrange(
                                "p (h x) -> p h x", x=DH),
                            op0=ALU.mult, op1=ALU.add)
            if "snT" in dbg:
                nc.sync.dma_start(dbg["snT"], snT[:])
            if "KT" in dbg:
                nc.sync.dma_start(dbg["KT"], KT[:])
            pool_snT.release()
            pool_qkv.release()

            # ================= mask =================
            need_mask = [any(plan[qp][kc] == 1 for qp in range(NQP))
                         for kc in range(KC)]
            mask = pool_mask.tile([128, KC, LQ], dt.bfloat16)
            nonneg = singles.tile([128, KC], dt.float32)
            nc.vector.tensor_scalar(out=nonneg[:], in0=modage_t[:], scalar1=0.0,
                                    scalar2=None, op0=ALU.is_ge)
            for kc in range(KC):
                if not need_mask[kc]:
                    continue
                nc.vector.tensor_scalar(out=mask[:, kc, :], in0=age_bc[:],
                                        scalar1=modage_t[:, kc:kc + 1],
                                        scalar2=None, op0=ALU.is_ge)
                nc.vector.tensor_scalar_mul(mask[:, kc, :], mask[:, kc, :],
                                            nonneg[:, kc:kc + 1])

            # ================= phase D: attention =================
            # Per-kc schedule shared by every head: merge the two query
            # windows into one wide op when both are live and have matching
            # accumulation state; per-window ops otherwise.
            last_live = [max((kc for kc in range(KC) if plan[qp][kc] != 0),
                             default=-1) for qp in range(NQP)]
            sched = []  # (kc, c0, c1, start, stop, mask_slices)
            seen = [False] * NQP
            for kc in range(KC):
                lv = [qp for qp in range(NQP) if plan[qp][kc] != 0]
                if not lv:
                    continue
                if len(lv) == 2 and seen[0] == seen[1]:
                    groups = [(0, 2 * QW, lv)]
                else:
                    groups = [(qp * QW, (qp + 1) * QW, [qp]) for qp in lv]
                for c0, c1, qps in groups:
                    msl = []
                    if all(plan[qp][kc] == 1 for qp in qps):
                        msl = [(c0, c1)]
                    else:
                        msl = [(qp * QW, (qp + 1) * QW) for qp in qps
                               if plan[qp][kc] == 1]
                    sched.append((kc, c0, c1, not seen[qps[0]],
                                  all(kc == last_live[qp] for qp in qps), msl))
                for qp in lv:
                    seen[qp] = True

            Yall = pool_yall.tile([128, EC, LQ], dt.bfloat16)
            with tc.tile_pool(name="phD", bufs=10) as phD, \
                 tc.tile_pool(name="phD_ps", bufs=2, space="PSUM") as phD_ps, \
                 tc.tile_pool(name="phD_psy", bufs=2, space="PSUM") as phD_psy:
                for hp in range(H // 2):
                    dc = hp
                    # both heads of the pair share one 2-bank psum + pt tile
                    py = phD_psy.tile([128, 2, LQ], dt.float32, space="PSUM",
                                      tag="py")
                    pts = []
                    for kc, c0, c1, st, sp, msl in sched:
                        w = c1 - c0
                        ps = phD_ps.tile([128, 2, 2 * QW], dt.float32,
                                         space="PSUM", tag="ps")
                        for hi in range(2):
                            nc.tensor.matmul(
                                ps[:, hi, 0:w],
                                KT[hi * DH:(hi + 1) * DH, dc,
                                   kc * 128:(kc + 1) * 128],
                                QT[hi * DH:(hi + 1) * DH, dc, c0:c1],
                                start=True, stop=True, skip_group_check=True)
                        pt = phD.tile([128, 2, 2 * QW], dt.bfloat16, tag="pt")
                        nc.scalar.activation(out=pt[:, :, 0:w], in_=ps[:, :, 0:w],
                                             func=AF.Exp, bias=0.0, scale=1.0)
                        for m0, m1 in msl:
                            mk = mask[:, kc, m0:m1]
                            mk2 = bass.AP(tensor=mk.tensor, offset=mk.offset,
                                          ap=[mk.ap[0], [0, 2], mk.ap[1]])
                            nc.vector.tensor_tensor(
                                out=pt[:, :, m0 - c0:m1 - c0],
                                in0=pt[:, :, m0 - c0:m1 - c0],
                                in1=mk2, op=ALU.mult)
                        pts.append(pt)
                    for hi in range(2):
                        h = 2 * hp + hi
                        for (kc, c0, c1, st, sp, msl), pt in zip(sched, pts):
                            nc.tensor.matmul(
                                py[0:DH + 1, hi, c0:c1],
                                Vaug[:, kc, h * (DH + 1):(h + 1) * (DH + 1)],
                                pt[:, hi, 0:c1 - c0],
                                start=st, stop=sp, skip_group_check=True)
                    for hi in range(2):
                        rec = phD.tile([1, LQ], dt.float32, tag="rec")
                        nc.vector.tensor_scalar_add(rec[:], py[DH:DH + 1, hi, :],
                                                    1e-30)
                        nc.vector.reciprocal_approx_fast(out=rec[:], in_=rec[:])
                        recb = phD.tile([DH, LQ], dt.float32, tag="recb")
                        nc.gpsimd.partition_broadcast(recb[:], rec[:])
                        nc.vector.tensor_mul(
                            out=Yall[hi * DH:(hi + 1) * DH, dc, :],
                            in0=py[0:DH, hi, :], in1=recb[:])
            if "Yall" in dbg:
                nc.sync.dma_start(dbg["Yall"], Yall[:])
            pool_mask.release()
            pool_qt.release()
            pool_kv.release()

            # ============ phase E: c-proj + residual, LN2 -> h1nT ============
            x2 = pool_x2.tile([128, QC, E], dt.float32)
            with tc.tile_pool(name="phE", bufs=1) as phE, \
                 tc.tile_pool(name="phE_ps", bufs=4, space="PSUM") as phE_ps:
                xe = phE.tile([128, QC, E], dt.float32, tag="xe")
                nc.sync.dma_start(xe[:], xh.rearrange("(c p) e -> p c e", p=128))
                for qc in range(QC):
                    for hf in range(2):
                        pc = phE_ps.tile([128, 384], dt.float32, space="PSUM",
                                         tag="pc")
                        for ec in range(EC):
                            nc.tensor.matmul(
                                pc[:], Yall[:, ec, qc * 128:(qc + 1) * 128],
                                cwT_t[:, ec, hf * 384:(hf + 1) * 384],
                                start=(ec == 0), stop=(ec == EC - 1))
                        sl = slice(hf * 384, (hf + 1) * 384)
                        nc.vector.scalar_tensor_tensor(
                            out=x2[:, qc, sl], in0=pc[:], scalar=1.0,
                            in1=cb_t[:, sl], op0=ALU.mult, op1=ALU.add)
                    nc.vector.tensor_add(out=x2[:, qc, :], in0=x2[:, qc, :],
                                         in1=xe[:, qc, :])
            if "x2" in dbg:
                nc.sync.dma_start(dbg["x2"], x2[:])

            pool_h1 = tc.alloc_tile_pool(name="pool_h1", bufs=1)
            h1nT = pool_h1.tile([128, EC, LQ], dt.bfloat16)
            with tc.tile_pool(name="phE2", bufs=2) as phE2, \
                 tc.tile_pool(name="phE2_ps", bufs=3, space="PSUM") as phE2_ps:
                layernorm_T(x2[:], QC, ln2w_t, ln2b_t, ln_ident[2], h1nT,
                            phE2, phE2_ps)

            # ================= phase F: MLP =================
            pool_hT = tc.alloc_tile_pool(name="pool_hT", bufs=1)
            hT = pool_hT.tile([128, FC, LQ], dt.bfloat16)
            with tc.tile_pool(name="phF", bufs=3) as phF, \
                 tc.tile_pool(name="phF_ps", bufs=2, space="PSUM") as phF_ps:
                for f in range(FC):
                    fw = phF.tile([128, EC, 128], dt.bfloat16, tag="fw")
                    nc.scalar.dma_start(fw[:], fcw_img[f]
                                      .rearrange("p (c x) -> p c x", c=EC))
                    ph = phF_ps.tile([128, LQ], dt.float32, space="PSUM", tag="ph")
                    for ec in range(EC):
                        nc.tensor.matmul(ph[:], fw[:, ec, :], h1nT[:, ec, :],
                                         start=(ec == 0), stop=(ec == EC - 1))
                    nc.scalar.activation(out=hT[:, f, :], in_=ph[:], func=AF.Gelu,
                                         bias=fcb_t[:, f:f + 1], scale=1.0)

            with tc.tile_pool(name="phG", bufs=3) as phG, \
                 tc.tile_pool(name="phG_ps", bufs=8, space="PSUM") as phG_ps, \
                 tc.tile_pool(name="phG_out", bufs=2) as phG_out:
                pps = [phG_ps.tile([128, 384], dt.float32, space="PSUM", tag="pp",
                                   name=f"pp{i}")
                       for i in range(2 * QC)]
                for f in range(FC):
                    pw = phG.tile([128, E], dt.bfloat16, tag="pw")
                    nc.scalar.dma_start(pw[:], pwT[f * 128:(f + 1) * 128, :])
                    for qc in range(QC):
                        for hf in range(2):
                            nc.tensor.matmul(
                                pps[qc * 2 + hf][:],
                                hT[:, f, qc * 128:(qc + 1) * 128],
                                pw[:, hf * 384:(hf + 1) * 384],
                                start=(f == 0), stop=(f == FC - 1),
                                skip_group_check=True)
                for qc in range(QC):
                    ot = phG_out.tile([128, E], dt.float32, tag="ot")
                    for hf in range(2):
                        sl = slice(hf * 384, (hf + 1) * 384)
                        nc.vector.scalar_tensor_tensor(
                            out=ot[:, sl], in0=pps[qc * 2 + hf][:], scalar=1.0,
                            in1=pb_t[:, sl], op0=ALU.mult, op1=ALU.add)
                    nc.vector.tensor_add(out=ot[:], in0=ot[:], in1=x2[:, qc, :])
                    nc.sync.dma_start(
                        out.rearrange("(c p) e -> p c e", p=128)[:, qc, :], ot[:])

            pool_hT.release()
            pool_h1.release()
            pool_x2.release()
            pool_yall.release()
            pool_cw.release()
            singles.release()

    nc.compile()
    return nc


def _to_img(wT):
    """[E, D] (e-major) -> SBUF image [128, EC*D]: img[p, c*D+d] = wT[c*128+p, d]."""
    Ei, D = wT.shape
    return np.ascontiguousarray(
        wT.reshape(Ei // 128, 128, D).transpose(1, 0, 2).reshape(128, -1))


def _host_prep(x, age, mod_idx, mod_age, mod2_emb, mod3_emb,
               ln0_w, ln0_b, ln1_w, ln1_b, ln2_w, ln2_b,
               q_w, q_b, k_w, k_b, v_w, v_b, c_w, c_b,
               fc_w, fc_b, proj_w, proj_b):
    f32 = np.float32
    x = np.asarray(x, f32); age = np.asarray(age, f32)
    mod_idx = np.asarray(mod_idx); mod_age = np.asarray(mod_age, f32)
    mod2_emb = np.asarray(mod2_emb, f32); mod3_emb = np.asarray(mod3_emb, f32)

    scale = np.float32(DH) ** -0.5
    qw_img = _to_img(np.asarray(q_w, f32).T * scale).astype(BF)
    kw_img = _to_img(np.asarray(k_w, f32).T).astype(BF)
    vw_img = _to_img(np.asarray(v_w, f32).T).astype(BF)
    cw_img = _to_img(np.asarray(c_w, f32).T).astype(BF)
    fcw_img = np.ascontiguousarray(
        np.asarray(fc_w, f32).T.reshape(EC, 128, FC, 128)
        .transpose(2, 1, 0, 3).reshape(FC, 128, E)).astype(BF)
    pwT = np.ascontiguousarray(np.asarray(proj_w, f32).T).astype(BF)
    qb2 = np.asarray(q_b, f32) * scale

    lnp = [np.asarray(a, f32) for a in
           (ln0_w, ln0_b, ln1_w, ln1_b, ln2_w, ln2_b)]
    ln_ident = tuple(
        bool(np.all(lnp[2 * i] == 1.0) and np.all(lnp[2 * i + 1] == 0.0))
        for i in range(3))

    shared = dict(
        qw_img=qw_img, kw_img=kw_img, vw_img=vw_img, cw_img=cw_img,
        fcw_img=fcw_img, pwT=pwT,
        qb=qb2, kb=np.asarray(k_b, f32), vb=np.asarray(v_b, f32),
        cb=np.asarray(c_b, f32), fcb=np.asarray(fc_b, f32),
        pb=np.asarray(proj_b, f32),
        ln0w=lnp[0], ln0b=lnp[1], ln1w=lnp[2], ln1b=lnp[3],
        ln2w=lnp[4], ln2b=lnp[5],
    )

    qrows = {0: np.array([t * 128 + i for t in QTILES_G0 for i in range(128)]),
             1: np.array([t * 128 + i for t in QTILES_G1 for i in range(128)])}

    in_maps = []
    plan_counts = np.zeros((NQP, KC, 3), dtype=np.int64)
    for core in range(N_CORES):
        b, g = core // 2, core % 2
        rows = qrows[g]
        order = np.argsort(mod_age[b], kind="stable")
        s_idx = np.asarray(mod_idx[b])[order]
        m2 = s_idx == 2
        m3 = s_idx == 3
        occ2 = np.clip(np.cumsum(m2) - 1, 0, HALF - 1)
        occ3 = np.clip(np.cumsum(m3) - 1, 0, HALF - 1)
        gi = np.full(LK, 2 * HALF, dtype=np.int32)
        gi[m2] = occ2[m2]
        gi[m3] = HALF + occ3[m3]
        embcat = np.concatenate([
            mod2_emb[b * HALF:(b + 1) * HALF],
            mod3_emb[b * HALF:(b + 1) * HALF],
            np.zeros((1, E), f32)], axis=0)
        agec = age[b][rows]
        mk = (agec[None, :] >= mod_age[b][:, None]) & (mod_age[b] >= 0.0)[:, None]
        for qp in range(NQP):
            sub = mk[:, qp * QW:(qp + 1) * QW]
            for kc in range(KC):
                blk = sub[kc * 128:(kc + 1) * 128]
                if not blk.any():
                    plan_counts[qp, kc, 0] += 1
                elif blk.all():
                    plan_counts[qp, kc, 2] += 1
                else:
                    plan_counts[qp, kc, 1] += 1
        in_maps.append(dict(
            xh=np.ascontiguousarray(x[b][rows]),
            embcat=embcat, gidx=gi,
            ageh=np.ascontiguousarray(agec),
            modage=np.ascontiguousarray(mod_age[b]),
            **shared))

    plan = []
    for qp in range(NQP):
        row = []
        for kc in range(KC):
            z, p, fl = plan_counts[qp, kc]
            if z == N_CORES:
                row.append(0)
            elif fl == N_CORES:
                row.append(2)
            else:
                row.append(1)
        if all(a == 0 for a in row):
            row[0] = 1
        plan.append(tuple(row))
    return in_maps, (tuple(plan), ln_ident), qrows


def _run(inputs, trace):
    in_maps, key, qrows = _host_prep(**inputs)
    if key not in _prog_cache:
        _prog_cache[key] = _build_program(*key)
    nc = _prog_cache[key]
    res = run_bass_kernel_spmd(nc, in_maps, core_ids=list(range(N_CORES)),
                               trace=trace)
    out = np.empty((B, 2 * LQ, E), dtype=np.float32)
    for core in range(N_CORES):
        b, g = core // 2, core % 2
        out[b, qrows[g]] = res.results[core]["out"]
    return out, res


def kernel(**inputs):
    return _run(inputs, trace=False)[0]


def run_traced(**inputs):
    return _run(inputs, trace=True)
